# revision 2
# baseline (speedup 1.0000x reference)
"""Multi-head self-attention (batch=2, seq=2048, embed=1024, heads=16, causal)
sharded over 8 NeuronCores: data-parallel over batch (x2) and tensor-parallel
over heads (x4 groups of 4 heads).

v2: software-pipelined attention windows (AV lags QK by 2 so the Scalar exp
latency never head-of-line-blocks the PE queue), diagonal-block column
trimming (exp/QK/AV/mask restricted to the causally-live column range),
projection matmuls interleaved into attention windows as PE fill, all of x
resident in SBUF, DMAs off the Scalar queue, PE warmup during startup DMAs.
"""

import os
from collections import deque

import ml_dtypes
import numpy as np
from contextlib import ExitStack

import concourse.bass as bass
import concourse.mybir as mybir
import concourse.tile as tile
from concourse import bacc
from concourse.bass_utils import run_bass_kernel_spmd

N_HEADS = 16
EMBED = 1024
HEAD = 64
SEQ = 2048
BATCH = 2
N_CORES = 8
HPC = 4                # heads per core
GCOLS = HPC * HEAD     # 256 embed columns per head group
P = 128
CH = 512               # seq chunk
NCH = SEQ // CH        # 4
KT = SEQ // P          # 16 k tiles
VW = HPC * (HEAD + 1)  # v row width per ktile (ones column at 64 per head)

DT = mybir.dt.float32
DTB = mybir.dt.bfloat16
DTH = mybir.dt.float16

LAST_EXEC_NS = None
LAST_RESULTS = None


def _build_program():
    nc = bacc.Bacc("TRN2", target_bir_lowering=False, debug=False,
                   num_devices=N_CORES)
    # All inputs host-packed into the exact SBUF-resident layouts so every
    # DMA is a contiguous per-partition run (hardware DGE at full rate).
    # xP[p, 4096*qi + 512*i + s]: x chunk-major; wqkF[p, 1024*f + 128*i + c];
    # wvF = wvbig layout; woF = wobig layout.
    xP = nc.dram_tensor("xP", [P, 8 * SEQ], DTB, kind="ExternalInput")
    wqkF = nc.dram_tensor("wqkF", [P, 4 * 8 * P], DTB, kind="ExternalInput")
    wvF = nc.dram_tensor("wvF", [P, 8 * GCOLS], DTB, kind="ExternalInput")
    bqk = nc.dram_tensor("bqk", [P, 4], DT, kind="ExternalInput")
    woF = nc.dram_tensor("woF", [P, 2 * EMBED], DTB, kind="ExternalInput")
    maskT = nc.dram_tensor("maskT", [P, CH], DTB, kind="ExternalInput")
    yT = nc.dram_tensor("yT", [EMBED, SEQ], DTH, kind="ExternalOutput")

    EXP = mybir.ActivationFunctionType.Exp

    with tile.TileContext(nc) as tc, ExitStack() as ctx:
        const = ctx.enter_context(tc.tile_pool(name="const", bufs=1))
        stpool = ctx.enter_context(tc.tile_pool(name="stpool", bufs=6))
        rpool = ctx.enter_context(tc.tile_pool(name="rpool", bufs=4))
        tailpool = ctx.enter_context(tc.tile_pool(name="tailpool", bufs=1))
        ypool = ctx.enter_context(tc.tile_pool(name="ypool", bufs=3))
        stP = ctx.enter_context(tc.tile_pool(name="stP", bufs=2, space="PSUM"))
        psO = ctx.enter_context(tc.tile_pool(name="psO", bufs=2, space="PSUM"))
        psP = ctx.enter_context(tc.tile_pool(name="psP", bufs=2, space="PSUM"))

        # ---- persistent SBUF residents ----
        wqkbig = const.tile([P, 8 * 2 * GCOLS], DTB, tag="wqkbig")
        wvbig = const.tile([P, 8 * GCOLS], DTB, tag="wvbig")
        wobig = const.tile([P, 2 * EMBED], DTB, tag="wobig")
        x_all = const.tile([P, 8 * SEQ], DTB, tag="xall")
        qt_t = [const.tile([P, SEQ], DTB, tag=f"qt{a}", name=f"qt{a}") for a in range(2)]
        kt_t = [const.tile([P, SEQ], DTB, tag=f"kt{a}", name=f"kt{a}") for a in range(2)]
        vtbig = const.tile([P, KT * VW], DTB, tag="vtbig")
        ohat_t = [const.tile([P, SEQ], DTB, tag=f"ohat{a}", name=f"ohat{a}") for a in range(2)]
        bqk_sb = const.tile([P, 4], DT, tag="bqk")
        mask_sb = const.tile([P, CH], DTB, tag="mask")

        # ---- PE warmup: burn the HAM cold window during the DMA wait.
        # Feed from a memset tile so the warmup is not gated on any DMA.
        wsrc = const.tile([P, HEAD], DTB, tag="wsrc")
        nc.gpsimd.memset(wsrc[:], 0.25)
        NWARM = 60
        wps = psP.tile([P, HEAD], DT, tag="psP", name="warm")
        for i in range(NWARM):
            nc.tensor.matmul(wps[0:HEAD, :], lhsT=wsrc[:, 0:HEAD],
                             rhs=wsrc[:, 0:HEAD],
                             start=(i == 0), stop=(i == NWARM - 1))
        wsc = rpool.tile([HEAD, HEAD], DT, tag="warmev", name="warmev")
        nc.vector.tensor_copy(wsc[:], wps[0:HEAD, :])

        # ---- preamble: gpsimd DMAs go through the slow software DGE
        # (~50 GB/s), so only tiny or late-needed transfers ride there; the
        # two hardware DGE queues (sync, scalar) carry everything that gates
        # the first qkv psets, in need-order.
        nc.gpsimd.dma_start(out=mask_sb, in_=maskT[:])
        nc.gpsimd.dma_start(out=bqk_sb, in_=bqk[:])
        nc.gpsimd.memset(
            vtbig.rearrange("p (t h d) -> p t h d",
                            t=KT, h=HPC)[:, :, :, HEAD:HEAD + 1], 1.0)

        def wqk_f(f):
            return bass.ds(8 * P * f, 8 * P)

        def xch(qi, half):
            return bass.ds(8 * CH * qi + 4 * CH * half, 4 * CH)

        nc.sync.dma_start(out=wqkbig[:, wqk_f(0)], in_=wqkF[:, wqk_f(0)])
        nc.scalar.dma_start(out=x_all[:, xch(0, 0)], in_=xP[:, xch(0, 0)])
        nc.scalar.dma_start(out=x_all[:, xch(0, 1)], in_=xP[:, xch(0, 1)])
        nc.sync.dma_start(out=wqkbig[:, wqk_f(1)], in_=wqkF[:, wqk_f(1)])
        nc.sync.dma_start(out=wqkbig[:, wqk_f(2)], in_=wqkF[:, wqk_f(2)])
        nc.scalar.dma_start(out=wqkbig[:, wqk_f(3)], in_=wqkF[:, wqk_f(3)])
        nc.sync.dma_start(out=wvbig[:, 0:4 * GCOLS], in_=wvF[:, 0:4 * GCOLS])
        nc.scalar.dma_start(out=wvbig[:, 4 * GCOLS:], in_=wvF[:, 4 * GCOLS:])
        for qi in range(1, NCH):
            nc.sync.dma_start(out=x_all[:, xch(qi, 0)], in_=xP[:, xch(qi, 0)])
            nc.scalar.dma_start(out=x_all[:, xch(qi, 1)], in_=xP[:, xch(qi, 1)])
        nc.gpsimd.dma_start(out=wobig[:], in_=woF[:])

        # ---- projection fill units ----
        # Each unit: (pe_ns_estimate, closure). Closures share per-pset state
        # so the accumulation psum tile is created at the first sub-unit.
        chunk_q = deque()   # qkv-chunk units: must complete before their pair
        oproj_q = deque()   # out-projection units: opportunistic

        def make_qk_pset(qi, f):
            sl = bass.ds(CH * qi, CH)
            st8 = {}

            def mm(i0, n):
                def go():
                    if "ps" not in st8:
                        st8["ps"] = psP.tile([P, CH], DT, tag="psP",
                                             name=f"psqk{qi}{f}")
                    ps = st8["ps"]
                    for i in range(i0, i0 + n):
                        nc.tensor.matmul(
                            ps,
                            lhsT=wqkbig[:, bass.ds(8 * P * f + P * i, P)],
                            rhs=x_all[:, bass.ds(8 * CH * qi + CH * i, CH)],
                            start=(i == 0), stop=(i == 7))
                return go

            def evac():
                dst = (qt_t if f < 2 else kt_t)[f % 2]
                nc.vector.tensor_scalar_add(dst[:, sl], st8["ps"],
                                            bqk_sb[:, f:f + 1])
            return [(450, mm(0, 2)), (450, mm(2, 2)), (450, mm(4, 2)),
                    (450, mm(6, 2)), (0, evac)]

        def make_v_pset(qi, s):
            ti = 4 * qi + s
            st8 = {}

            def mm(i0, n):
                def go():
                    if "ps" not in st8:
                        st8["ps"] = psP.tile([P, GCOLS], DT, tag="psP",
                                             name=f"psv{qi}{s}")
                    ps = st8["ps"]
                    for i in range(i0, i0 + n):
                        nc.tensor.matmul(
                            ps,
                            lhsT=x_all[:, bass.ds(8 * CH * qi + CH * i + P * s, P)],
                            rhs=wvbig[:, bass.ds(GCOLS * i, GCOLS)],
                            start=(i == 0), stop=(i == 7))
                return go

            def evac():
                dst = vtbig[:, bass.ds(VW * ti, VW)].rearrange(
                    "p (h d) -> p h d", h=HPC)[:, :, 0:HEAD]
                nc.vector.tensor_copy(dst, st8["ps"].rearrange(
                    "p (h d) -> p h d", h=HPC))
            return [(460, mm(0, 4)), (460, mm(4, 4)), (0, evac)]

        def make_o_pset(qi, m, eng, pool=None, ev=None):
            sl = bass.ds(CH * qi, CH)
            st8 = {}
            pool = pool or psP

            def mm():
                if pool is stP:
                    t = pool.tile([P, 2 * CH], DT, tag="stP", name=f"pso{qi}{m}")
                    st8["ps"] = t[:, 0:CH]
                else:
                    st8["ps"] = pool.tile([P, CH], DT, tag="psP",
                                          name=f"pso{qi}{m}")
                for k in range(2):
                    nc.tensor.matmul(
                        st8["ps"],
                        lhsT=wobig[:, bass.ds(EMBED * k + P * m, P)],
                        rhs=ohat_t[k][:, sl],
                        start=(k == 0), stop=(k == 1))

            def evac():
                ysb = ypool.tile([P, CH], DTH, tag="ysb", name=f"ysb{qi}{m}")
                if ev is nc.scalar:
                    nc.scalar.copy(ysb[:], st8["ps"])
                else:
                    nc.vector.tensor_copy(ysb[:], st8["ps"])
                eng.dma_start(
                    out=yT.rearrange("(m p) s -> p m s", p=P)[:, m, sl],
                    in_=ysb[:])
            return [(470, mm), (0, evac)]

        def queue_qkv_chunk(qi):
            for f in range(4):
                chunk_q.extend(make_qk_pset(qi, f))
            for s in range(4):
                chunk_q.extend(make_v_pset(qi, s))

        def queue_oproj(qi, pool=None):
            for m in range(8):
                if pool is stP:
                    # tail: keep the output DMAs on the fast hardware DGEs
                    eng = nc.sync if m % 2 == 0 else nc.scalar
                    ev = nc.scalar if m % 2 == 1 else None
                else:
                    eng = nc.sync if m % 2 == 0 else nc.gpsimd
                    ev = None
                oproj_q.extend(make_o_pset(qi, m, eng, pool=pool, ev=ev))

        def fill(budget):
            while budget > 0 and (chunk_q or oproj_q):
                q = chunk_q if chunk_q else oproj_q
                cost, fn = q.popleft()
                fn()
                budget -= max(cost, 120)

        def flush(q):
            while q:
                _, fn = q.popleft()
                fn()

        # ---- attention pair: pipelined window loop ----
        LAG = 2

        def attn_pair(hp, qi):
            nk = 4 * qi + 4
            po = [psO.tile([P, CH], DT, tag="psO", name=f"po{hp}{qi}{hh}")
                  for hh in range(2)]
            sts = {}
            for w in range(nk + LAG):
                ki = w
                if ki < nk:
                    kr = ki - 4 * qi
                    trim = P * kr if kr >= 0 else 0
                    W = CH - trim
                    pst = stP.tile([P, 2 * CH], DT, tag="stP", name="pst")
                    for hh in range(2):
                        r0 = HEAD * hh
                        nc.tensor.matmul(
                            pst[:, bass.ds(CH * hh + trim, W)],
                            lhsT=kt_t[hp][r0:r0 + HEAD, bass.ds(P * ki, P)],
                            rhs=qt_t[hp][r0:r0 + HEAD, bass.ds(CH * qi + trim, W)],
                            start=True, stop=True)
                    st = stpool.tile([P, 2 * CH], DTB, tag="st", name="st")
                    psrc = pst.rearrange("p (h c) -> p h c", h=2)[:, :, trim:CH]
                    sdst = st.rearrange("p (h c) -> p h c", h=2)[:, :, trim:CH]
                    nc.scalar.activation(sdst, psrc, EXP, scale=0.125)
                    if kr >= 0:
                        for hh in range(2):
                            nc.vector.tensor_mul(
                                st[:, bass.ds(CH * hh + trim, W)],
                                st[:, bass.ds(CH * hh + trim, W)],
                                mask_sb[:, 0:W])
                    sts[ki] = (st, trim)
                    act_ns = (2 * W + 352) * 0.833
                else:
                    act_ns = 1200
                # AV lags QK by LAG windows; the first AV slips one extra
                # window (caught up at w=LAG+1) so the previous pair's
                # normalize has freed the psO ring before AV[0] queues.
                if w in (LAG, LAG + 1):
                    kavs = []
                elif w == LAG + 2:
                    kavs = [0, 1, 2][:nk]
                else:
                    kavs = [w - LAG]
                av_ns = 0
                for kav in kavs:
                    if not (0 <= kav < nk):
                        continue
                    st, trim = sts.pop(kav)
                    W = CH - trim
                    for hh in range(2):
                        h = 2 * hp + hh
                        nc.tensor.matmul(
                            po[hh][0:HEAD + 1, trim:CH],
                            lhsT=vtbig[:, bass.ds(VW * kav + (HEAD + 1) * h,
                                                  HEAD + 1)],
                            rhs=st[:, bass.ds(CH * hh + trim, W)],
                            start=(kav == 0), stop=(kav == nk - 1))
                    av_ns = 2 * (W * 0.42 + 40)
                qk_ns = (CH - max(0, ki - 4 * qi) * P) * 0.42 + 60 if ki < nk else 0
                fill(act_ns - qk_ns - av_ns + 150)
            # normalize: recip of the ones-row denominator, broadcast, scale.
            # On the last pair the outproj tail waits on this chain: copy the
            # denominators on the (idle) scalar engine and process in column
            # halves so the second half's chain is all that trails the last AV.
            last = (hp == 1 and qi == NCH - 1)
            if not last:
                for hh in range(2):
                    r0 = HEAD * hh
                    den = rpool.tile([1, CH], DT, tag=f"den{hh}", name="den")
                    nc.vector.tensor_copy(den[:], po[hh][HEAD:HEAD + 1, :])
                    recip = rpool.tile([1, CH], DT, tag="recip", name="recip")
                    nc.vector.reciprocal_approx_fast(recip[:], den[:])
                    recipb = rpool.tile([HEAD, CH], DT, tag="recipb", name="recipb")
                    nc.gpsimd.partition_broadcast(recipb[:], recip[:])
                    nc.vector.tensor_mul(
                        ohat_t[hp][r0:r0 + HEAD, bass.ds(CH * qi, CH)],
                        po[hh][0:HEAD, :], recipb[:])
            else:
                # tail chain: fresh tiles (no ring-reuse waits), scalar copies,
                # heads pipelined across scalar/DVE/gpsimd
                dens, recips = [], []
                for hh in range(2):
                    den = tailpool.tile([1, CH], DT, tag=f"tden{hh}", name="tden")
                    nc.scalar.copy(den[:], po[hh][HEAD:HEAD + 1, :])
                    dens.append(den)
                for hh in range(2):
                    recip = tailpool.tile([1, CH], DT, tag=f"trec{hh}", name="trec")
                    nc.vector.reciprocal_approx_fast(recip[:], dens[hh][:])
                    recips.append(recip)
                for hh in range(2):
                    r0 = HEAD * hh
                    recipb = tailpool.tile([HEAD, CH], DT, tag=f"trb{hh}", name="trb")
                    nc.gpsimd.partition_broadcast(recipb[:], recips[hh][:])
                    nc.vector.tensor_mul(
                        ohat_t[hp][r0:r0 + HEAD, bass.ds(CH * qi, CH)],
                        po[hh][0:HEAD, :], recipb[:])

        # ---- schedule ----
        queue_qkv_chunk(0)
        flush(chunk_q)
        queue_qkv_chunk(1)
        attn_pair(0, 0)
        attn_pair(1, 0)
        flush(chunk_q)
        queue_qkv_chunk(2)
        attn_pair(0, 1)
        attn_pair(1, 1)
        flush(chunk_q)
        queue_qkv_chunk(3)
        queue_oproj(0)
        attn_pair(0, 2)
        attn_pair(1, 2)
        flush(chunk_q)
        queue_oproj(1)
        queue_oproj(2)
        attn_pair(0, 3)
        attn_pair(1, 3)
        flush(oproj_q)
        queue_oproj(3, pool=stP)
        flush(oproj_q)

    nc.compile()
    return nc


def _make_mask():
    p = np.arange(P)[:, None]
    j = np.arange(CH)[None, :]
    return (p <= j).astype(np.float32)


def kernel(x, W_qkv, b_qkv, W_out, b_out):
    global LAST_EXEC_NS, LAST_RESULTS
    x = np.asarray(x, dtype=np.float32)
    W_qkv = np.asarray(W_qkv, dtype=np.float32)
    b_qkv = np.asarray(b_qkv, dtype=np.float32)
    W_out = np.asarray(W_out, dtype=np.float32)
    b_out = np.asarray(b_out, dtype=np.float32)

    nc = _build_program()
    mask = _make_mask()

    in_maps = []
    for c in range(N_CORES):
        b, g = divmod(c, HPC)
        q0 = GCOLS * g
        wq = W_qkv[q0:q0 + GCOLS]                    # [256, 1024]
        wk = W_qkv[EMBED + q0:EMBED + q0 + GCOLS]
        wv = W_qkv[2 * EMBED + q0:2 * EMBED + q0 + GCOLS]
        bq = b_qkv[q0:q0 + GCOLS]
        bk = b_qkv[EMBED + q0:EMBED + q0 + GCOLS]
        bqk = np.stack([bq[0:P], bq[P:2 * P], bk[0:P], bk[P:2 * P]],
                       axis=1).astype(np.float32)   # [128, 4]
        wqkt = np.concatenate([wq, wk], 0).T            # [1024 embed, 512]
        wqkf = wqkt.reshape(8, P, 4, P).transpose(1, 2, 0, 3).reshape(P, 4 * 8 * P)
        # xP[p, qi, i, s]: chunk-major so each chunk is one contiguous DMA
        xp = x[b].T.reshape(8, P, NCH, CH).transpose(1, 2, 0, 3).reshape(P, 8 * SEQ)
        # wvF in wvbig layout [p, i, 256]
        wvf = wv.T.reshape(8, P, GCOLS).transpose(1, 0, 2).reshape(P, 8 * GCOLS)
        # woF in wobig layout [p, k, 1024]
        wof = W_out[:, q0:q0 + GCOLS].T.reshape(2, P, EMBED).transpose(
            1, 0, 2).reshape(P, 2 * EMBED)
        in_maps.append({
            "xP": np.ascontiguousarray(xp).astype(ml_dtypes.bfloat16),
            "wqkF": np.ascontiguousarray(wqkf).astype(ml_dtypes.bfloat16),
            "wvF": np.ascontiguousarray(wvf).astype(ml_dtypes.bfloat16),
            "bqk": np.ascontiguousarray(bqk),
            "woF": np.ascontiguousarray(wof).astype(ml_dtypes.bfloat16),
            "maskT": mask.astype(ml_dtypes.bfloat16),
        })

    want_trace = bool(int(os.environ.get("KTRACE", "0")))
    if want_trace:
        try:
            import antenv.axon_hooks  # noqa: F401
        except ImportError:
            want_trace = False
    res = run_bass_kernel_spmd(nc, in_maps, list(range(N_CORES)),
                               trace=want_trace,
                               tmpdir=os.environ.get("KTRACE_DIR") or None)
    LAST_EXEC_NS = res.exec_time_ns
    LAST_RESULTS = res

    out = np.empty((BATCH, SEQ, EMBED), dtype=np.float32)
    crow = (b_out + W_out @ b_qkv[2 * EMBED:]).astype(np.float32)
    for b in range(BATCH):
        acc = np.zeros((EMBED, SEQ), dtype=np.float32)
        for g in range(HPC):
            acc += res.results[HPC * b + g]["yT"].astype(np.float32)
        out[b] = acc.T + crow[None, :]
    return out


# revision 3
# speedup vs baseline: 1.0065x; 1.0065x over previous
"""Multi-head self-attention (batch=2, seq=2048, embed=1024, heads=16, causal)
sharded over 8 NeuronCores: data-parallel over batch (x2) and tensor-parallel
over heads (x4 groups of 4 heads).

v2: software-pipelined attention windows (AV lags QK by 2 so the Scalar exp
latency never head-of-line-blocks the PE queue), diagonal-block column
trimming (exp/QK/AV/mask restricted to the causally-live column range),
projection matmuls interleaved into attention windows as PE fill, all of x
resident in SBUF, DMAs off the Scalar queue, PE warmup during startup DMAs.
"""

import os
from collections import deque

import ml_dtypes
import numpy as np
from contextlib import ExitStack

import concourse.bass as bass
import concourse.mybir as mybir
import concourse.tile as tile
from concourse import bacc
from concourse.bass_utils import run_bass_kernel_spmd

N_HEADS = 16
EMBED = 1024
HEAD = 64
SEQ = 2048
BATCH = 2
N_CORES = 8
HPC = 4                # heads per core
GCOLS = HPC * HEAD     # 256 embed columns per head group
P = 128
CH = 512               # seq chunk
NCH = SEQ // CH        # 4
KT = SEQ // P          # 16 k tiles
VW = HPC * (HEAD + 1)  # v row width per ktile (ones column at 64 per head)

DT = mybir.dt.float32
DTB = mybir.dt.bfloat16
DTH = mybir.dt.float16

LAST_EXEC_NS = None
LAST_RESULTS = None


def _build_program():
    nc = bacc.Bacc("TRN2", target_bir_lowering=False, debug=False,
                   num_devices=N_CORES)
    # All inputs host-packed into the exact SBUF-resident layouts so every
    # DMA is a contiguous per-partition run (hardware DGE at full rate).
    # xP[p, 4096*qi + 512*i + s]: x chunk-major; wqkF[p, 1024*f + 128*i + c];
    # wvF = wvbig layout; woF = wobig layout.
    xP = nc.dram_tensor("xP", [P, 8 * SEQ], DTB, kind="ExternalInput")
    wqkF = nc.dram_tensor("wqkF", [P, 4 * 8 * P], DTB, kind="ExternalInput")
    wvF = nc.dram_tensor("wvF", [P, 8 * GCOLS], DTB, kind="ExternalInput")
    bqk = nc.dram_tensor("bqk", [P, 4], DT, kind="ExternalInput")
    woF = nc.dram_tensor("woF", [P, 2 * EMBED], DTB, kind="ExternalInput")
    maskT = nc.dram_tensor("maskT", [P, CH], DTB, kind="ExternalInput")
    yT = nc.dram_tensor("yT", [EMBED, SEQ], DTH, kind="ExternalOutput")

    EXP = mybir.ActivationFunctionType.Exp

    with tile.TileContext(nc) as tc, ExitStack() as ctx:
        const = ctx.enter_context(tc.tile_pool(name="const", bufs=1))
        stpool = ctx.enter_context(tc.tile_pool(name="stpool", bufs=6))
        rpool = ctx.enter_context(tc.tile_pool(name="rpool", bufs=4))
        tailpool = ctx.enter_context(tc.tile_pool(name="tailpool", bufs=1))
        ypool = ctx.enter_context(tc.tile_pool(name="ypool", bufs=4))
        stP = ctx.enter_context(tc.tile_pool(name="stP", bufs=2, space="PSUM"))
        psO = ctx.enter_context(tc.tile_pool(name="psO", bufs=2, space="PSUM"))
        psP = ctx.enter_context(tc.tile_pool(name="psP", bufs=2, space="PSUM"))

        # ---- persistent SBUF residents ----
        wqkbig = const.tile([P, 8 * 2 * GCOLS], DTB, tag="wqkbig")
        wvbig = const.tile([P, 8 * GCOLS], DTB, tag="wvbig")
        wobig = const.tile([P, 2 * EMBED], DTB, tag="wobig")
        x_all = const.tile([P, 8 * SEQ], DTB, tag="xall")
        qt_t = [const.tile([P, SEQ], DTB, tag=f"qt{a}", name=f"qt{a}") for a in range(2)]
        kt_t = [const.tile([P, SEQ], DTB, tag=f"kt{a}", name=f"kt{a}") for a in range(2)]
        vtbig = const.tile([P, KT * VW], DTB, tag="vtbig")
        ohat_t = [const.tile([P, SEQ], DTB, tag=f"ohat{a}", name=f"ohat{a}") for a in range(2)]
        bqk_sb = const.tile([P, 4], DT, tag="bqk")
        mask_sb = const.tile([P, CH], DTB, tag="mask")

        # ---- PE warmup: burn the HAM cold window during the DMA wait.
        # Feed from a memset tile so the warmup is not gated on any DMA.
        wsrc = const.tile([P, HEAD], DTB, tag="wsrc")
        nc.gpsimd.memset(wsrc[:], 0.25)
        NWARM = 60
        wps = psP.tile([P, HEAD], DT, tag="psP", name="warm")
        for i in range(NWARM):
            nc.tensor.matmul(wps[0:HEAD, :], lhsT=wsrc[:, 0:HEAD],
                             rhs=wsrc[:, 0:HEAD],
                             start=(i == 0), stop=(i == NWARM - 1))
        wsc = rpool.tile([HEAD, HEAD], DT, tag="warmev", name="warmev")
        nc.vector.tensor_copy(wsc[:], wps[0:HEAD, :])

        # ---- preamble: gpsimd DMAs go through the slow software DGE
        # (~50 GB/s), so only tiny or late-needed transfers ride there; the
        # two hardware DGE queues (sync, scalar) carry everything that gates
        # the first qkv psets, in need-order.
        nc.gpsimd.dma_start(out=mask_sb, in_=maskT[:])
        nc.gpsimd.dma_start(out=bqk_sb, in_=bqk[:])
        nc.gpsimd.memset(
            vtbig.rearrange("p (t h d) -> p t h d",
                            t=KT, h=HPC)[:, :, :, HEAD:HEAD + 1], 1.0)

        def wqk_f(f):
            return bass.ds(8 * P * f, 8 * P)

        def xch(qi, half):
            return bass.ds(8 * CH * qi + 4 * CH * half, 4 * CH)

        def xq(qi, quarter):
            return bass.ds(8 * CH * qi + 2 * CH * quarter, 2 * CH)

        nc.sync.dma_start(out=wqkbig[:, wqk_f(0)], in_=wqkF[:, wqk_f(0)])
        nc.scalar.dma_start(out=x_all[:, xq(0, 0)], in_=xP[:, xq(0, 0)])
        nc.scalar.dma_start(out=x_all[:, xq(0, 1)], in_=xP[:, xq(0, 1)])
        nc.sync.dma_start(out=x_all[:, xq(0, 2)], in_=xP[:, xq(0, 2)])
        nc.sync.dma_start(out=x_all[:, xq(0, 3)], in_=xP[:, xq(0, 3)])
        nc.scalar.dma_start(out=wqkbig[:, wqk_f(1)], in_=wqkF[:, wqk_f(1)])
        nc.sync.dma_start(out=wqkbig[:, wqk_f(2)], in_=wqkF[:, wqk_f(2)])
        nc.scalar.dma_start(out=wqkbig[:, wqk_f(3)], in_=wqkF[:, wqk_f(3)])
        nc.sync.dma_start(out=wvbig[:, 0:4 * GCOLS], in_=wvF[:, 0:4 * GCOLS])
        nc.scalar.dma_start(out=wvbig[:, 4 * GCOLS:], in_=wvF[:, 4 * GCOLS:])
        for qi in range(1, NCH):
            nc.sync.dma_start(out=x_all[:, xch(qi, 0)], in_=xP[:, xch(qi, 0)])
            nc.scalar.dma_start(out=x_all[:, xch(qi, 1)], in_=xP[:, xch(qi, 1)])
        nc.gpsimd.dma_start(out=wobig[:], in_=woF[:])

        # ---- projection fill units ----
        # Each unit: (pe_ns_estimate, closure). Closures share per-pset state
        # so the accumulation psum tile is created at the first sub-unit.
        chunk_q = deque()   # qkv-chunk units: must complete before their pair
        oproj_q = deque()   # out-projection units: opportunistic

        def make_qk_pset(qi, f):
            sl = bass.ds(CH * qi, CH)
            st8 = {}

            def mm(i0, n):
                def go():
                    if "ps" not in st8:
                        st8["ps"] = psP.tile([P, CH], DT, tag="psP",
                                             name=f"psqk{qi}{f}")
                    ps = st8["ps"]
                    for i in range(i0, i0 + n):
                        nc.tensor.matmul(
                            ps,
                            lhsT=wqkbig[:, bass.ds(8 * P * f + P * i, P)],
                            rhs=x_all[:, bass.ds(8 * CH * qi + CH * i, CH)],
                            start=(i == 0), stop=(i == 7))
                return go

            def evac():
                dst = (qt_t if f < 2 else kt_t)[f % 2]
                nc.vector.tensor_scalar_add(dst[:, sl], st8["ps"],
                                            bqk_sb[:, f:f + 1])
            return [(450, mm(0, 2)), (450, mm(2, 2)), (450, mm(4, 2)),
                    (450, mm(6, 2)), (0, evac)]

        def make_v_pset(qi, s):
            ti = 4 * qi + s
            st8 = {}

            def mm(i0, n):
                def go():
                    if "ps" not in st8:
                        st8["ps"] = psP.tile([P, GCOLS], DT, tag="psP",
                                             name=f"psv{qi}{s}")
                    ps = st8["ps"]
                    for i in range(i0, i0 + n):
                        nc.tensor.matmul(
                            ps,
                            lhsT=x_all[:, bass.ds(8 * CH * qi + CH * i + P * s, P)],
                            rhs=wvbig[:, bass.ds(GCOLS * i, GCOLS)],
                            start=(i == 0), stop=(i == 7))
                return go

            def evac():
                dst = vtbig[:, bass.ds(VW * ti, VW)].rearrange(
                    "p (h d) -> p h d", h=HPC)[:, :, 0:HEAD]
                nc.vector.tensor_copy(dst, st8["ps"].rearrange(
                    "p (h d) -> p h d", h=HPC))
            return [(460, mm(0, 4)), (460, mm(4, 4)), (0, evac)]

        def make_o_pset(qi, m, eng, pool=None, ev=None):
            sl = bass.ds(CH * qi, CH)
            st8 = {}
            pool = pool or psP

            def mm():
                if pool is stP:
                    t = pool.tile([P, 2 * CH], DT, tag="stP", name=f"pso{qi}{m}")
                    st8["ps"] = t[:, 0:CH]
                else:
                    st8["ps"] = pool.tile([P, CH], DT, tag="psP",
                                          name=f"pso{qi}{m}")
                for k in range(2):
                    nc.tensor.matmul(
                        st8["ps"],
                        lhsT=wobig[:, bass.ds(EMBED * k + P * m, P)],
                        rhs=ohat_t[k][:, sl],
                        start=(k == 0), stop=(k == 1))

            def evac():
                ysb = ypool.tile([P, CH], DTH, tag="ysb", name=f"ysb{qi}{m}")
                if ev is nc.scalar:
                    nc.scalar.copy(ysb[:], st8["ps"])
                else:
                    nc.vector.tensor_copy(ysb[:], st8["ps"])
                eng.dma_start(
                    out=yT.rearrange("(m p) s -> p m s", p=P)[:, m, sl],
                    in_=ysb[:])
            return [(470, mm), (0, evac)]

        def queue_qkv_chunk(qi):
            for f in range(4):
                chunk_q.extend(make_qk_pset(qi, f))
            for s in range(4):
                chunk_q.extend(make_v_pset(qi, s))

        def queue_oproj(qi, pool=None):
            for m in range(8):
                if pool is stP:
                    # tail: keep the output DMAs on the fast hardware DGEs
                    eng = nc.sync if m % 2 == 0 else nc.scalar
                    ev = nc.scalar if m % 2 == 1 else None
                else:
                    eng = nc.sync if m % 2 == 0 else nc.gpsimd
                    ev = None
                oproj_q.extend(make_o_pset(qi, m, eng, pool=pool, ev=ev))

        def fill(budget):
            while budget > 0 and (chunk_q or oproj_q):
                q = chunk_q if chunk_q else oproj_q
                cost, fn = q.popleft()
                fn()
                budget -= max(cost, 120)

        def flush(q):
            while q:
                _, fn = q.popleft()
                fn()

        # ---- attention pair: pipelined window loop ----
        LAG = 2

        def attn_pair(hp, qi):
            nk = 4 * qi + 4
            po = [psO.tile([P, CH], DT, tag="psO", name=f"po{hp}{qi}{hh}")
                  for hh in range(2)]
            sts = {}
            for w in range(nk + LAG):
                ki = w
                if ki < nk:
                    kr = ki - 4 * qi
                    trim = P * kr if kr >= 0 else 0
                    W = CH - trim
                    pst = stP.tile([P, 2 * CH], DT, tag="stP", name="pst")
                    for hh in range(2):
                        r0 = HEAD * hh
                        nc.tensor.matmul(
                            pst[:, bass.ds(CH * hh + trim, W)],
                            lhsT=kt_t[hp][r0:r0 + HEAD, bass.ds(P * ki, P)],
                            rhs=qt_t[hp][r0:r0 + HEAD, bass.ds(CH * qi + trim, W)],
                            start=True, stop=True)
                    st = stpool.tile([P, 2 * CH], DTB, tag="st", name="st")
                    psrc = pst.rearrange("p (h c) -> p h c", h=2)[:, :, trim:CH]
                    sdst = st.rearrange("p (h c) -> p h c", h=2)[:, :, trim:CH]
                    nc.scalar.activation(sdst, psrc, EXP, scale=0.125)
                    if kr >= 0:
                        for hh in range(2):
                            nc.vector.tensor_mul(
                                st[:, bass.ds(CH * hh + trim, W)],
                                st[:, bass.ds(CH * hh + trim, W)],
                                mask_sb[:, 0:W])
                    sts[ki] = (st, trim)
                    act_ns = (2 * W + 352) * 0.833
                else:
                    act_ns = 1200
                # AV lags QK by LAG windows; the first AV slips one extra
                # window (caught up at w=LAG+1) so the previous pair's
                # normalize has freed the psO ring before AV[0] queues.
                if w in (LAG, LAG + 1):
                    kavs = []
                elif w == LAG + 2:
                    kavs = [0, 1, 2][:nk]
                else:
                    kavs = [w - LAG]
                av_ns = 0
                for kav in kavs:
                    if not (0 <= kav < nk):
                        continue
                    st, trim = sts.pop(kav)
                    W = CH - trim
                    for hh in range(2):
                        h = 2 * hp + hh
                        nc.tensor.matmul(
                            po[hh][0:HEAD + 1, trim:CH],
                            lhsT=vtbig[:, bass.ds(VW * kav + (HEAD + 1) * h,
                                                  HEAD + 1)],
                            rhs=st[:, bass.ds(CH * hh + trim, W)],
                            start=(kav == 0), stop=(kav == nk - 1))
                    av_ns = 2 * (W * 0.42 + 40)
                qk_ns = (CH - max(0, ki - 4 * qi) * P) * 0.42 + 60 if ki < nk else 0
                fill(act_ns - qk_ns - av_ns + 150)
            # normalize: recip of the ones-row denominator, broadcast, scale.
            # On the last pair the outproj tail waits on this chain: copy the
            # denominators on the (idle) scalar engine and process in column
            # halves so the second half's chain is all that trails the last AV.
            last = (hp == 1 and qi == NCH - 1)
            if not last:
                for hh in range(2):
                    r0 = HEAD * hh
                    den = rpool.tile([1, CH], DT, tag=f"den{hh}", name="den")
                    nc.vector.tensor_copy(den[:], po[hh][HEAD:HEAD + 1, :])
                    recip = rpool.tile([1, CH], DT, tag="recip", name="recip")
                    nc.vector.reciprocal_approx_fast(recip[:], den[:])
                    recipb = rpool.tile([HEAD, CH], DT, tag="recipb", name="recipb")
                    nc.gpsimd.partition_broadcast(recipb[:], recip[:])
                    nc.vector.tensor_mul(
                        ohat_t[hp][r0:r0 + HEAD, bass.ds(CH * qi, CH)],
                        po[hh][0:HEAD, :], recipb[:])
            else:
                # tail chain: fresh tiles (no ring-reuse waits), scalar copies,
                # heads pipelined across scalar/DVE/gpsimd
                dens, recips = [], []
                for hh in range(2):
                    den = tailpool.tile([1, CH], DT, tag=f"tden{hh}", name="tden")
                    nc.scalar.copy(den[:], po[hh][HEAD:HEAD + 1, :])
                    dens.append(den)
                for hh in range(2):
                    recip = tailpool.tile([1, CH], DT, tag=f"trec{hh}", name="trec")
                    nc.vector.reciprocal_approx_fast(recip[:], dens[hh][:])
                    recips.append(recip)
                for hh in range(2):
                    r0 = HEAD * hh
                    recipb = tailpool.tile([HEAD, CH], DT, tag=f"trb{hh}", name="trb")
                    nc.gpsimd.partition_broadcast(recipb[:], recips[hh][:])
                    nc.vector.tensor_mul(
                        ohat_t[hp][r0:r0 + HEAD, bass.ds(CH * qi, CH)],
                        po[hh][0:HEAD, :], recipb[:])

        # ---- schedule ----
        queue_qkv_chunk(0)
        flush(chunk_q)
        queue_qkv_chunk(1)
        attn_pair(0, 0)
        attn_pair(1, 0)
        flush(chunk_q)
        queue_qkv_chunk(2)
        attn_pair(0, 1)
        attn_pair(1, 1)
        flush(chunk_q)
        queue_qkv_chunk(3)
        queue_oproj(0)
        attn_pair(0, 2)
        attn_pair(1, 2)
        flush(chunk_q)
        queue_oproj(1)
        queue_oproj(2)
        attn_pair(0, 3)
        attn_pair(1, 3)
        flush(oproj_q)
        queue_oproj(3, pool=stP)
        flush(oproj_q)

    nc.compile()
    return nc


def _make_mask():
    p = np.arange(P)[:, None]
    j = np.arange(CH)[None, :]
    return (p <= j).astype(np.float32)


def kernel(x, W_qkv, b_qkv, W_out, b_out):
    global LAST_EXEC_NS, LAST_RESULTS
    x = np.asarray(x, dtype=np.float32)
    W_qkv = np.asarray(W_qkv, dtype=np.float32)
    b_qkv = np.asarray(b_qkv, dtype=np.float32)
    W_out = np.asarray(W_out, dtype=np.float32)
    b_out = np.asarray(b_out, dtype=np.float32)

    nc = _build_program()
    mask = _make_mask()

    in_maps = []
    for c in range(N_CORES):
        b, g = divmod(c, HPC)
        q0 = GCOLS * g
        wq = W_qkv[q0:q0 + GCOLS]                    # [256, 1024]
        wk = W_qkv[EMBED + q0:EMBED + q0 + GCOLS]
        wv = W_qkv[2 * EMBED + q0:2 * EMBED + q0 + GCOLS]
        bq = b_qkv[q0:q0 + GCOLS]
        bk = b_qkv[EMBED + q0:EMBED + q0 + GCOLS]
        bqk = np.stack([bq[0:P], bq[P:2 * P], bk[0:P], bk[P:2 * P]],
                       axis=1).astype(np.float32)   # [128, 4]
        wqkt = np.concatenate([wq, wk], 0).T            # [1024 embed, 512]
        wqkf = wqkt.reshape(8, P, 4, P).transpose(1, 2, 0, 3).reshape(P, 4 * 8 * P)
        # xP[p, qi, i, s]: chunk-major so each chunk is one contiguous DMA
        xp = x[b].T.reshape(8, P, NCH, CH).transpose(1, 2, 0, 3).reshape(P, 8 * SEQ)
        # wvF in wvbig layout [p, i, 256]
        wvf = wv.T.reshape(8, P, GCOLS).transpose(1, 0, 2).reshape(P, 8 * GCOLS)
        # woF in wobig layout [p, k, 1024]
        wof = W_out[:, q0:q0 + GCOLS].T.reshape(2, P, EMBED).transpose(
            1, 0, 2).reshape(P, 2 * EMBED)
        in_maps.append({
            "xP": np.ascontiguousarray(xp).astype(ml_dtypes.bfloat16),
            "wqkF": np.ascontiguousarray(wqkf).astype(ml_dtypes.bfloat16),
            "wvF": np.ascontiguousarray(wvf).astype(ml_dtypes.bfloat16),
            "bqk": np.ascontiguousarray(bqk),
            "woF": np.ascontiguousarray(wof).astype(ml_dtypes.bfloat16),
            "maskT": mask.astype(ml_dtypes.bfloat16),
        })

    want_trace = bool(int(os.environ.get("KTRACE", "0")))
    if want_trace:
        try:
            import antenv.axon_hooks  # noqa: F401
        except ImportError:
            want_trace = False
    res = run_bass_kernel_spmd(nc, in_maps, list(range(N_CORES)),
                               trace=want_trace,
                               tmpdir=os.environ.get("KTRACE_DIR") or None)
    LAST_EXEC_NS = res.exec_time_ns
    LAST_RESULTS = res

    out = np.empty((BATCH, SEQ, EMBED), dtype=np.float32)
    crow = (b_out + W_out @ b_qkv[2 * EMBED:]).astype(np.float32)
    for b in range(BATCH):
        acc = np.zeros((EMBED, SEQ), dtype=np.float32)
        for g in range(HPC):
            acc += res.results[HPC * b + g]["yT"].astype(np.float32)
        out[b] = acc.T + crow[None, :]
    return out


# revision 4
# speedup vs baseline: 1.0109x; 1.0043x over previous
"""Multi-head self-attention (batch=2, seq=2048, embed=1024, heads=16, causal)
sharded over 8 NeuronCores: data-parallel over batch (x2) and tensor-parallel
over heads (x4 groups of 4 heads).

Per core: qkvT projection with the bias folded into the psum evacuation,
row-tiled QK^T (the two 64-contraction head matmuls run concurrently on the
PE), causal softmax with the denominator folded into the AV matmul via a
ones-column on V, and a partial output projection; the host sums the four
partials per batch and adds the constant row b_out + W_out @ b_v.

Schedule: software-pipelined attention windows (AV lags QK by 2 windows so
the Scalar-engine exp latency never head-of-line-blocks the in-order PE
queue), diagonal-block column trimming (QK/exp/mask/AV restricted to the
causally-live column range), projection matmuls interleaved into the
attention windows as PE fill with double-buffered psum psets, all of x
resident in SBUF, inputs host-packed to make every DMA contiguous on the
hardware DGE queues (gpsimd's software DGE only carries tiny/late
transfers), and PE warmup matmuls burning the HAM cold window during the
startup DMAs.
"""

import os
from collections import deque

import ml_dtypes
import numpy as np
from contextlib import ExitStack

import concourse.bass as bass
import concourse.mybir as mybir
import concourse.tile as tile
from concourse import bacc
from concourse.bass_utils import run_bass_kernel_spmd

N_HEADS = 16
EMBED = 1024
HEAD = 64
SEQ = 2048
BATCH = 2
N_CORES = 8
HPC = 4                # heads per core
GCOLS = HPC * HEAD     # 256 embed columns per head group
P = 128
CH = 512               # seq chunk
NCH = SEQ // CH        # 4
KT = SEQ // P          # 16 k tiles
VW = HPC * (HEAD + 1)  # v row width per ktile (ones column at 64 per head)

DT = mybir.dt.float32
DTB = mybir.dt.bfloat16
DTH = mybir.dt.float16

LAST_EXEC_NS = None
LAST_RESULTS = None


def _build_program():
    nc = bacc.Bacc("TRN2", target_bir_lowering=False, debug=False,
                   num_devices=N_CORES)
    # All inputs host-packed into the exact SBUF-resident layouts so every
    # DMA is a contiguous per-partition run (hardware DGE at full rate).
    # xP[p, 4096*qi + 512*i + s]: x chunk-major; wqkF[p, 1024*f + 128*i + c];
    # wvF = wvbig layout; woF = wobig layout.
    xP = nc.dram_tensor("xP", [P, 8 * SEQ], DTB, kind="ExternalInput")
    wqkF = nc.dram_tensor("wqkF", [P, 4 * 8 * P], DTB, kind="ExternalInput")
    wvF = nc.dram_tensor("wvF", [P, 8 * GCOLS], DTB, kind="ExternalInput")
    bqk = nc.dram_tensor("bqk", [P, 4], DT, kind="ExternalInput")
    woF = nc.dram_tensor("woF", [P, 2 * EMBED], DTB, kind="ExternalInput")
    maskT = nc.dram_tensor("maskT", [P, CH], DTB, kind="ExternalInput")
    yT = nc.dram_tensor("yT", [EMBED, SEQ], DTH, kind="ExternalOutput")

    EXP = mybir.ActivationFunctionType.Exp

    with tile.TileContext(nc) as tc, ExitStack() as ctx:
        const = ctx.enter_context(tc.tile_pool(name="const", bufs=1))
        stpool = ctx.enter_context(tc.tile_pool(name="stpool", bufs=6))
        rpool = ctx.enter_context(tc.tile_pool(name="rpool", bufs=4))
        tailpool = ctx.enter_context(tc.tile_pool(name="tailpool", bufs=1))
        ypool = ctx.enter_context(tc.tile_pool(name="ypool", bufs=4))
        stP = ctx.enter_context(tc.tile_pool(name="stP", bufs=2, space="PSUM"))
        psO = ctx.enter_context(tc.tile_pool(name="psO", bufs=2, space="PSUM"))
        psP = ctx.enter_context(tc.tile_pool(name="psP", bufs=2, space="PSUM"))

        # ---- persistent SBUF residents ----
        wqkbig = const.tile([P, 8 * 2 * GCOLS], DTB, tag="wqkbig")
        wvbig = const.tile([P, 8 * GCOLS], DTB, tag="wvbig")
        wobig = const.tile([P, 2 * EMBED], DTB, tag="wobig")
        x_all = const.tile([P, 8 * SEQ], DTB, tag="xall")
        qt_t = [const.tile([P, SEQ], DTB, tag=f"qt{a}", name=f"qt{a}") for a in range(2)]
        kt_t = [const.tile([P, SEQ], DTB, tag=f"kt{a}", name=f"kt{a}") for a in range(2)]
        vtbig = const.tile([P, KT * VW], DTB, tag="vtbig")
        ohat_t = [const.tile([P, SEQ], DTB, tag=f"ohat{a}", name=f"ohat{a}") for a in range(2)]
        bqk_sb = const.tile([P, 4], DT, tag="bqk")
        mask_sb = const.tile([P, CH], DTB, tag="mask")

        # ---- PE warmup: burn the HAM cold window during the DMA wait.
        # Feed from a memset tile so the warmup is not gated on any DMA.
        wsrc = const.tile([P, HEAD], DTB, tag="wsrc")
        nc.gpsimd.memset(wsrc[:], 0.25)
        NWARM = 60
        wps = psP.tile([P, HEAD], DT, tag="psP", name="warm")
        for i in range(NWARM):
            nc.tensor.matmul(wps[0:HEAD, :], lhsT=wsrc[:, 0:HEAD],
                             rhs=wsrc[:, 0:HEAD],
                             start=(i == 0), stop=(i == NWARM - 1))
        wsc = rpool.tile([HEAD, HEAD], DT, tag="warmev", name="warmev")
        nc.vector.tensor_copy(wsc[:], wps[0:HEAD, :])

        # ---- preamble: gpsimd DMAs go through the slow software DGE
        # (~50 GB/s), so only tiny or late-needed transfers ride there; the
        # two hardware DGE queues (sync, scalar) carry everything that gates
        # the first qkv psets, in need-order.
        nc.gpsimd.dma_start(out=mask_sb, in_=maskT[:])
        nc.gpsimd.dma_start(out=bqk_sb, in_=bqk[:])
        nc.gpsimd.memset(
            vtbig.rearrange("p (t h d) -> p t h d",
                            t=KT, h=HPC)[:, :, :, HEAD:HEAD + 1], 1.0)

        def wqk_f(f):
            return bass.ds(8 * P * f, 8 * P)

        def xch(qi, half):
            return bass.ds(8 * CH * qi + 4 * CH * half, 4 * CH)

        def xq(qi, quarter):
            return bass.ds(8 * CH * qi + 2 * CH * quarter, 2 * CH)

        nc.sync.dma_start(out=wqkbig[:, wqk_f(0)], in_=wqkF[:, wqk_f(0)])
        nc.scalar.dma_start(out=x_all[:, xq(0, 0)], in_=xP[:, xq(0, 0)])
        nc.scalar.dma_start(out=x_all[:, xq(0, 1)], in_=xP[:, xq(0, 1)])
        nc.sync.dma_start(out=x_all[:, xq(0, 2)], in_=xP[:, xq(0, 2)])
        nc.sync.dma_start(out=x_all[:, xq(0, 3)], in_=xP[:, xq(0, 3)])
        nc.scalar.dma_start(out=wqkbig[:, wqk_f(1)], in_=wqkF[:, wqk_f(1)])
        nc.sync.dma_start(out=wqkbig[:, wqk_f(2)], in_=wqkF[:, wqk_f(2)])
        nc.scalar.dma_start(out=wqkbig[:, wqk_f(3)], in_=wqkF[:, wqk_f(3)])
        nc.sync.dma_start(out=wvbig[:, 0:4 * GCOLS], in_=wvF[:, 0:4 * GCOLS])
        nc.scalar.dma_start(out=wvbig[:, 4 * GCOLS:], in_=wvF[:, 4 * GCOLS:])
        for qi in range(1, NCH):
            nc.sync.dma_start(out=x_all[:, xch(qi, 0)], in_=xP[:, xch(qi, 0)])
            nc.scalar.dma_start(out=x_all[:, xch(qi, 1)], in_=xP[:, xch(qi, 1)])
        nc.gpsimd.dma_start(out=wobig[:], in_=woF[:])

        # ---- projection fill units ----
        # Each unit: (pe_ns_estimate, closure). Closures share per-pset state
        # so the accumulation psum tile is created at the first sub-unit.
        chunk_q = deque()   # qkv-chunk units: must complete before their pair
        oproj_q = deque()   # out-projection units: opportunistic

        def make_qk_pset(qi, f):
            sl = bass.ds(CH * qi, CH)
            st8 = {}

            def mm(i0, n):
                def go():
                    if "ps" not in st8:
                        st8["ps"] = psP.tile([P, CH], DT, tag="psP",
                                             name=f"psqk{qi}{f}")
                    ps = st8["ps"]
                    for i in range(i0, i0 + n):
                        nc.tensor.matmul(
                            ps,
                            lhsT=wqkbig[:, bass.ds(8 * P * f + P * i, P)],
                            rhs=x_all[:, bass.ds(8 * CH * qi + CH * i, CH)],
                            start=(i == 0), stop=(i == 7))
                return go

            def evac():
                dst = (qt_t if f < 2 else kt_t)[f % 2]
                nc.vector.tensor_scalar_add(dst[:, sl], st8["ps"],
                                            bqk_sb[:, f:f + 1])
            return [(450, mm(0, 2)), (450, mm(2, 2)), (450, mm(4, 2)),
                    (450, mm(6, 2)), (0, evac)]

        def make_v_pset(qi, s):
            ti = 4 * qi + s
            st8 = {}

            def mm(i0, n):
                def go():
                    if "ps" not in st8:
                        st8["ps"] = psP.tile([P, GCOLS], DT, tag="psP",
                                             name=f"psv{qi}{s}")
                    ps = st8["ps"]
                    for i in range(i0, i0 + n):
                        nc.tensor.matmul(
                            ps,
                            lhsT=x_all[:, bass.ds(8 * CH * qi + CH * i + P * s, P)],
                            rhs=wvbig[:, bass.ds(GCOLS * i, GCOLS)],
                            start=(i == 0), stop=(i == 7))
                return go

            def evac():
                dst = vtbig[:, bass.ds(VW * ti, VW)].rearrange(
                    "p (h d) -> p h d", h=HPC)[:, :, 0:HEAD]
                nc.vector.tensor_copy(dst, st8["ps"].rearrange(
                    "p (h d) -> p h d", h=HPC))
            return [(460, mm(0, 4)), (460, mm(4, 4)), (0, evac)]

        def make_o_pset(qi, m, eng, pool=None, ev=None):
            sl = bass.ds(CH * qi, CH)
            st8 = {}
            pool = pool or psP

            def mm():
                if pool is stP:
                    t = pool.tile([P, 2 * CH], DT, tag="stP", name=f"pso{qi}{m}")
                    st8["ps"] = t[:, 0:CH]
                else:
                    st8["ps"] = pool.tile([P, CH], DT, tag="psP",
                                          name=f"pso{qi}{m}")
                for k in range(2):
                    nc.tensor.matmul(
                        st8["ps"],
                        lhsT=wobig[:, bass.ds(EMBED * k + P * m, P)],
                        rhs=ohat_t[k][:, sl],
                        start=(k == 0), stop=(k == 1))

            def evac():
                ysb = ypool.tile([P, CH], DTH, tag="ysb", name=f"ysb{qi}{m}")
                if ev is nc.scalar:
                    nc.scalar.copy(ysb[:], st8["ps"])
                else:
                    nc.vector.tensor_copy(ysb[:], st8["ps"])
                eng.dma_start(
                    out=yT.rearrange("(m p) s -> p m s", p=P)[:, m, sl],
                    in_=ysb[:])
            return [(470, mm), (0, evac)]

        def queue_qkv_chunk(qi):
            for f in range(4):
                chunk_q.extend(make_qk_pset(qi, f))
            for s in range(4):
                chunk_q.extend(make_v_pset(qi, s))

        def queue_oproj(qi, pool=None):
            for m in range(8):
                if pool is stP:
                    # tail: keep the output DMAs on the fast hardware DGEs
                    eng = nc.sync if m % 2 == 0 else nc.scalar
                    ev = nc.scalar if m % 2 == 1 else None
                else:
                    eng = nc.sync if m % 2 == 0 else nc.gpsimd
                    ev = None
                oproj_q.extend(make_o_pset(qi, m, eng, pool=pool, ev=ev))

        def fill(budget):
            while budget > 0 and (chunk_q or oproj_q):
                q = chunk_q if chunk_q else oproj_q
                cost, fn = q.popleft()
                fn()
                budget -= max(cost, 120)

        def flush(q):
            while q:
                _, fn = q.popleft()
                fn()

        # ---- attention pair: pipelined window loop ----
        LAG = 2

        def attn_pair(hp, qi):
            nk = 4 * qi + 4
            po = [psO.tile([P, CH], DT, tag="psO", name=f"po{hp}{qi}{hh}")
                  for hh in range(2)]
            sts = {}
            for w in range(nk + LAG):
                ki = w
                if ki < nk:
                    kr = ki - 4 * qi
                    trim = P * kr if kr >= 0 else 0
                    W = CH - trim
                    pst = stP.tile([P, 2 * CH], DT, tag="stP", name="pst")
                    for hh in range(2):
                        r0 = HEAD * hh
                        nc.tensor.matmul(
                            pst[:, bass.ds(CH * hh + trim, W)],
                            lhsT=kt_t[hp][r0:r0 + HEAD, bass.ds(P * ki, P)],
                            rhs=qt_t[hp][r0:r0 + HEAD, bass.ds(CH * qi + trim, W)],
                            start=True, stop=True)
                    st = stpool.tile([P, 2 * CH], DTB, tag="st", name="st")
                    psrc = pst.rearrange("p (h c) -> p h c", h=2)[:, :, trim:CH]
                    sdst = st.rearrange("p (h c) -> p h c", h=2)[:, :, trim:CH]
                    nc.scalar.activation(sdst, psrc, EXP, scale=0.125)
                    if kr >= 0:
                        for hh in range(2):
                            nc.vector.tensor_mul(
                                st[:, bass.ds(CH * hh + trim, W)],
                                st[:, bass.ds(CH * hh + trim, W)],
                                mask_sb[:, 0:W])
                    sts[ki] = (st, trim)
                    act_ns = (2 * W + 352) * 0.833
                else:
                    act_ns = 1200
                # AV lags QK by LAG windows; the first AV slips one extra
                # window (caught up at w=LAG+1) so the previous pair's
                # normalize has freed the psO ring before AV[0] queues.
                if w in (LAG, LAG + 1):
                    kavs = []
                elif w == LAG + 2:
                    kavs = [0, 1, 2][:nk]
                else:
                    kavs = [w - LAG]
                av_ns = 0
                for kav in kavs:
                    if not (0 <= kav < nk):
                        continue
                    st, trim = sts.pop(kav)
                    W = CH - trim
                    for hh in range(2):
                        h = 2 * hp + hh
                        nc.tensor.matmul(
                            po[hh][0:HEAD + 1, trim:CH],
                            lhsT=vtbig[:, bass.ds(VW * kav + (HEAD + 1) * h,
                                                  HEAD + 1)],
                            rhs=st[:, bass.ds(CH * hh + trim, W)],
                            start=(kav == 0), stop=(kav == nk - 1))
                    av_ns = 2 * (W * 0.42 + 40)
                qk_ns = (CH - max(0, ki - 4 * qi) * P) * 0.42 + 60 if ki < nk else 0
                fill(act_ns - qk_ns - av_ns + 150)
            # normalize: recip of the ones-row denominator, broadcast, scale.
            # On the last pair the outproj tail waits on this chain: copy the
            # denominators on the (idle) scalar engine and process in column
            # halves so the second half's chain is all that trails the last AV.
            last = (hp == 1 and qi == NCH - 1)
            if not last:
                for hh in range(2):
                    r0 = HEAD * hh
                    den = rpool.tile([1, CH], DT, tag=f"den{hh}", name="den")
                    nc.vector.tensor_copy(den[:], po[hh][HEAD:HEAD + 1, :])
                    recip = rpool.tile([1, CH], DT, tag="recip", name="recip")
                    nc.vector.reciprocal_approx_fast(recip[:], den[:])
                    recipb = rpool.tile([HEAD, CH], DT, tag="recipb", name="recipb")
                    nc.gpsimd.partition_broadcast(recipb[:], recip[:])
                    nc.vector.tensor_mul(
                        ohat_t[hp][r0:r0 + HEAD, bass.ds(CH * qi, CH)],
                        po[hh][0:HEAD, :], recipb[:])
            else:
                # tail chain: fresh tiles (no ring-reuse waits), scalar copies,
                # heads pipelined across scalar/DVE/gpsimd
                dens, recips = [], []
                for hh in range(2):
                    den = tailpool.tile([1, CH], DT, tag=f"tden{hh}", name="tden")
                    nc.scalar.copy(den[:], po[hh][HEAD:HEAD + 1, :])
                    dens.append(den)
                for hh in range(2):
                    recip = tailpool.tile([1, CH], DT, tag=f"trec{hh}", name="trec")
                    nc.vector.reciprocal_approx_fast(recip[:], dens[hh][:])
                    recips.append(recip)
                for hh in range(2):
                    r0 = HEAD * hh
                    recipb = tailpool.tile([HEAD, CH], DT, tag=f"trb{hh}", name="trb")
                    nc.gpsimd.partition_broadcast(recipb[:], recips[hh][:])
                    nc.vector.tensor_mul(
                        ohat_t[hp][r0:r0 + HEAD, bass.ds(CH * qi, CH)],
                        po[hh][0:HEAD, :], recipb[:])

        # ---- schedule ----
        queue_qkv_chunk(0)
        flush(chunk_q)
        queue_qkv_chunk(1)
        attn_pair(0, 0)
        attn_pair(1, 0)
        flush(chunk_q)
        queue_qkv_chunk(2)
        attn_pair(0, 1)
        attn_pair(1, 1)
        flush(chunk_q)
        queue_qkv_chunk(3)
        queue_oproj(0)
        attn_pair(0, 2)
        attn_pair(1, 2)
        flush(chunk_q)
        queue_oproj(1)
        queue_oproj(2)
        attn_pair(0, 3)
        attn_pair(1, 3)
        flush(oproj_q)
        queue_oproj(3, pool=stP)
        flush(oproj_q)

    nc.compile()
    return nc


def _make_mask():
    p = np.arange(P)[:, None]
    j = np.arange(CH)[None, :]
    return (p <= j).astype(np.float32)


def kernel(x, W_qkv, b_qkv, W_out, b_out):
    global LAST_EXEC_NS, LAST_RESULTS
    x = np.asarray(x, dtype=np.float32)
    W_qkv = np.asarray(W_qkv, dtype=np.float32)
    b_qkv = np.asarray(b_qkv, dtype=np.float32)
    W_out = np.asarray(W_out, dtype=np.float32)
    b_out = np.asarray(b_out, dtype=np.float32)

    nc = _build_program()
    mask = _make_mask()

    in_maps = []
    for c in range(N_CORES):
        b, g = divmod(c, HPC)
        q0 = GCOLS * g
        wq = W_qkv[q0:q0 + GCOLS]                    # [256, 1024]
        wk = W_qkv[EMBED + q0:EMBED + q0 + GCOLS]
        wv = W_qkv[2 * EMBED + q0:2 * EMBED + q0 + GCOLS]
        bq = b_qkv[q0:q0 + GCOLS]
        bk = b_qkv[EMBED + q0:EMBED + q0 + GCOLS]
        bqk = np.stack([bq[0:P], bq[P:2 * P], bk[0:P], bk[P:2 * P]],
                       axis=1).astype(np.float32)   # [128, 4]
        wqkt = np.concatenate([wq, wk], 0).T            # [1024 embed, 512]
        wqkf = wqkt.reshape(8, P, 4, P).transpose(1, 2, 0, 3).reshape(P, 4 * 8 * P)
        # xP[p, qi, i, s]: chunk-major so each chunk is one contiguous DMA
        xp = x[b].T.reshape(8, P, NCH, CH).transpose(1, 2, 0, 3).reshape(P, 8 * SEQ)
        # wvF in wvbig layout [p, i, 256]
        wvf = wv.T.reshape(8, P, GCOLS).transpose(1, 0, 2).reshape(P, 8 * GCOLS)
        # woF in wobig layout [p, k, 1024]
        wof = W_out[:, q0:q0 + GCOLS].T.reshape(2, P, EMBED).transpose(
            1, 0, 2).reshape(P, 2 * EMBED)
        in_maps.append({
            "xP": np.ascontiguousarray(xp).astype(ml_dtypes.bfloat16),
            "wqkF": np.ascontiguousarray(wqkf).astype(ml_dtypes.bfloat16),
            "wvF": np.ascontiguousarray(wvf).astype(ml_dtypes.bfloat16),
            "bqk": np.ascontiguousarray(bqk),
            "woF": np.ascontiguousarray(wof).astype(ml_dtypes.bfloat16),
            "maskT": mask.astype(ml_dtypes.bfloat16),
        })

    want_trace = bool(int(os.environ.get("KTRACE", "0")))
    if want_trace:
        try:
            import antenv.axon_hooks  # noqa: F401
        except ImportError:
            want_trace = False
    res = run_bass_kernel_spmd(nc, in_maps, list(range(N_CORES)),
                               trace=want_trace,
                               tmpdir=os.environ.get("KTRACE_DIR") or None)
    LAST_EXEC_NS = res.exec_time_ns
    LAST_RESULTS = res

    out = np.empty((BATCH, SEQ, EMBED), dtype=np.float32)
    crow = (b_out + W_out @ b_qkv[2 * EMBED:]).astype(np.float32)
    for b in range(BATCH):
        acc = np.zeros((EMBED, SEQ), dtype=np.float32)
        for g in range(HPC):
            acc += res.results[HPC * b + g]["yT"].astype(np.float32)
        out[b] = acc.T + crow[None, :]
    return out


# revision 5
# speedup vs baseline: 1.0227x; 1.0117x over previous
"""Multi-head self-attention (batch=2, seq=2048, embed=1024, heads=16, causal)
sharded over 8 NeuronCores: data-parallel over batch (x2) and tensor-parallel
over heads (x4 groups of 4 heads).

Per core: qkvT projection with the bias folded into the psum evacuation,
row-tiled QK^T (the two 64-contraction head matmuls run concurrently on the
PE), causal softmax with the denominator folded into the AV matmul via a
ones-column on V, and a partial output projection; the host sums the four
partials per batch and adds the constant row b_out + W_out @ b_v.

Schedule: software-pipelined attention windows (AV lags QK by 2 windows so
the Scalar-engine exp latency never head-of-line-blocks the in-order PE
queue), diagonal-block column trimming (QK/exp/mask/AV restricted to the
causally-live column range), projection matmuls interleaved into the
attention windows as PE fill with double-buffered psum psets, all of x
resident in SBUF, inputs host-packed to make every DMA contiguous on the
hardware DGE queues (gpsimd's software DGE only carries tiny/late
transfers), and PE warmup matmuls burning the HAM cold window during the
startup DMAs.
"""

import os
from collections import deque

import ml_dtypes
import numpy as np
from contextlib import ExitStack

import concourse.bass as bass
import concourse.mybir as mybir
import concourse.tile as tile
from concourse import bacc
from concourse.bass_utils import run_bass_kernel_spmd

N_HEADS = 16
EMBED = 1024
HEAD = 64
SEQ = 2048
BATCH = 2
N_CORES = 8
HPC = 4                # heads per core
GCOLS = HPC * HEAD     # 256 embed columns per head group
P = 128
CH = 512               # seq chunk
NCH = SEQ // CH        # 4
KT = SEQ // P          # 16 k tiles
VW = HPC * (HEAD + 1)  # v row width per ktile (ones column at 64 per head)

DT = mybir.dt.float32
DTB = mybir.dt.bfloat16
DTH = mybir.dt.float16

LAST_EXEC_NS = None
LAST_RESULTS = None


def _build_program():
    nc = bacc.Bacc("TRN2", target_bir_lowering=False, debug=False,
                   num_devices=N_CORES)
    # All inputs host-packed into the exact SBUF-resident layouts so every
    # DMA is a contiguous per-partition run (hardware DGE at full rate).
    # xP[p, 4096*qi + 512*i + s]: x chunk-major; wqkF[p, 1024*f + 128*i + c];
    # wvF = wvbig layout; woF = wobig layout.
    xP = nc.dram_tensor("xP", [P, 8 * SEQ], DTB, kind="ExternalInput")
    wqkF = nc.dram_tensor("wqkF", [P, 4 * 8 * P], DTB, kind="ExternalInput")
    wvF = nc.dram_tensor("wvF", [P, 8 * GCOLS], DTB, kind="ExternalInput")
    bqk = nc.dram_tensor("bqk", [P, 4], DT, kind="ExternalInput")
    woF = nc.dram_tensor("woF", [P, 2 * EMBED], DTB, kind="ExternalInput")
    maskT = nc.dram_tensor("maskT", [P, CH], DTB, kind="ExternalInput")
    yT = nc.dram_tensor("yT", [EMBED, SEQ], DTH, kind="ExternalOutput")

    EXP = mybir.ActivationFunctionType.Exp

    with tile.TileContext(nc) as tc, ExitStack() as ctx:
        const = ctx.enter_context(tc.tile_pool(name="const", bufs=1))
        stpool = ctx.enter_context(tc.tile_pool(name="stpool", bufs=6))
        rpool = ctx.enter_context(tc.tile_pool(name="rpool", bufs=4))
        tailpool = ctx.enter_context(tc.tile_pool(name="tailpool", bufs=1))
        ypool = ctx.enter_context(tc.tile_pool(name="ypool", bufs=4))
        stP = ctx.enter_context(tc.tile_pool(name="stP", bufs=2, space="PSUM"))
        psO = ctx.enter_context(tc.tile_pool(name="psO", bufs=2, space="PSUM"))
        psP = ctx.enter_context(tc.tile_pool(name="psP", bufs=2, space="PSUM"))

        # ---- persistent SBUF residents ----
        wqkbig = const.tile([P, 8 * 2 * GCOLS], DTB, tag="wqkbig")
        wvbig = const.tile([P, 8 * GCOLS], DTB, tag="wvbig")
        wobig = const.tile([P, 2 * EMBED], DTB, tag="wobig")
        x_all = const.tile([P, 8 * SEQ], DTB, tag="xall")
        qt_t = [const.tile([P, SEQ], DTB, tag=f"qt{a}", name=f"qt{a}") for a in range(2)]
        kt_t = [const.tile([P, SEQ], DTB, tag=f"kt{a}", name=f"kt{a}") for a in range(2)]
        vtbig = const.tile([P, KT * VW], DTB, tag="vtbig")
        ohat_t = [const.tile([P, SEQ], DTB, tag=f"ohat{a}", name=f"ohat{a}") for a in range(2)]
        bqk_sb = const.tile([P, 4], DT, tag="bqk")
        mask_sb = const.tile([P, CH], DTB, tag="mask")

        # ---- PE warmup: burn the HAM cold window during the DMA wait.
        # Feed from a memset tile so the warmup is not gated on any DMA.
        wsrc = const.tile([P, HEAD], DTB, tag="wsrc")
        nc.gpsimd.memset(wsrc[:], 0.25)
        NWARM = 60
        wps = psP.tile([P, HEAD], DT, tag="psP", name="warm")
        for i in range(NWARM):
            nc.tensor.matmul(wps[0:HEAD, :], lhsT=wsrc[:, 0:HEAD],
                             rhs=wsrc[:, 0:HEAD],
                             start=(i == 0), stop=(i == NWARM - 1))
        wsc = rpool.tile([HEAD, HEAD], DT, tag="warmev", name="warmev")
        nc.vector.tensor_copy(wsc[:], wps[0:HEAD, :])

        # ---- preamble: gpsimd DMAs go through the slow software DGE
        # (~50 GB/s), so only tiny or late-needed transfers ride there; the
        # two hardware DGE queues (sync, scalar) carry everything that gates
        # the first qkv psets, in need-order.
        nc.gpsimd.dma_start(out=mask_sb, in_=maskT[:])
        nc.gpsimd.dma_start(out=bqk_sb, in_=bqk[:])
        nc.gpsimd.memset(
            vtbig.rearrange("p (t h d) -> p t h d",
                            t=KT, h=HPC)[:, :, :, HEAD:HEAD + 1], 1.0)

        def wqk_f(f):
            return bass.ds(8 * P * f, 8 * P)

        def xch(qi, half):
            return bass.ds(8 * CH * qi + 4 * CH * half, 4 * CH)

        def xq(qi, quarter):
            return bass.ds(8 * CH * qi + 2 * CH * quarter, 2 * CH)

        nc.sync.dma_start(out=wqkbig[:, bass.ds(0, 4 * P)],
                          in_=wqkF[:, bass.ds(0, 4 * P)])
        nc.scalar.dma_start(out=wqkbig[:, bass.ds(4 * P, 4 * P)],
                            in_=wqkF[:, bass.ds(4 * P, 4 * P)])
        nc.scalar.dma_start(out=x_all[:, xq(0, 0)], in_=xP[:, xq(0, 0)])
        nc.sync.dma_start(out=x_all[:, xq(0, 1)], in_=xP[:, xq(0, 1)])
        nc.scalar.dma_start(out=x_all[:, xq(0, 2)], in_=xP[:, xq(0, 2)])
        nc.sync.dma_start(out=x_all[:, xq(0, 3)], in_=xP[:, xq(0, 3)])
        nc.scalar.dma_start(out=wqkbig[:, wqk_f(1)], in_=wqkF[:, wqk_f(1)])
        nc.sync.dma_start(out=wqkbig[:, wqk_f(2)], in_=wqkF[:, wqk_f(2)])
        nc.scalar.dma_start(out=wqkbig[:, wqk_f(3)], in_=wqkF[:, wqk_f(3)])
        nc.sync.dma_start(out=wvbig[:, 0:4 * GCOLS], in_=wvF[:, 0:4 * GCOLS])
        nc.scalar.dma_start(out=wvbig[:, 4 * GCOLS:], in_=wvF[:, 4 * GCOLS:])
        for qi in range(1, NCH):
            nc.sync.dma_start(out=x_all[:, xch(qi, 0)], in_=xP[:, xch(qi, 0)])
            nc.scalar.dma_start(out=x_all[:, xch(qi, 1)], in_=xP[:, xch(qi, 1)])
        nc.gpsimd.dma_start(out=wobig[:], in_=woF[:])

        # ---- projection fill units ----
        # Each unit: (pe_ns_estimate, closure). Closures share per-pset state
        # so the accumulation psum tile is created at the first sub-unit.
        chunk_q = deque()   # qkv-chunk units: must complete before their pair
        oproj_q = deque()   # out-projection units: opportunistic

        def make_qk_pset(qi, f):
            sl = bass.ds(CH * qi, CH)
            st8 = {}

            def mm(c, i0, n):
                # N=256 half-column chains pipeline at ~120ns/MM where the
                # N=512 chains pay a ~63ns/MM bubble
                def go():
                    if "ps" not in st8:
                        st8["ps"] = psP.tile([P, CH], DT, tag="psP",
                                             name=f"psqk{qi}{f}")
                    ps = st8["ps"]
                    for i in range(i0, i0 + n):
                        nc.tensor.matmul(
                            ps[:, bass.ds(256 * c, 256)],
                            lhsT=wqkbig[:, bass.ds(8 * P * f + P * i, P)],
                            rhs=x_all[:, bass.ds(8 * CH * qi + CH * i + 256 * c,
                                                 256)],
                            start=(i == 0), stop=(i == 7))
                return go

            def evac():
                dst = (qt_t if f < 2 else kt_t)[f % 2]
                nc.vector.tensor_scalar_add(dst[:, sl], st8["ps"],
                                            bqk_sb[:, f:f + 1])
            return [(520, mm(0, 0, 4)), (520, mm(0, 4, 4)),
                    (520, mm(1, 0, 4)), (520, mm(1, 4, 4)), (0, evac)]

        def make_v_pset(qi, s):
            ti = 4 * qi + s
            st8 = {}

            def mm(i0, n):
                def go():
                    if "ps" not in st8:
                        st8["ps"] = psP.tile([P, GCOLS], DT, tag="psP",
                                             name=f"psv{qi}{s}")
                    ps = st8["ps"]
                    for i in range(i0, i0 + n):
                        nc.tensor.matmul(
                            ps,
                            lhsT=x_all[:, bass.ds(8 * CH * qi + CH * i + P * s, P)],
                            rhs=wvbig[:, bass.ds(GCOLS * i, GCOLS)],
                            start=(i == 0), stop=(i == 7))
                return go

            def evac():
                dst = vtbig[:, bass.ds(VW * ti, VW)].rearrange(
                    "p (h d) -> p h d", h=HPC)[:, :, 0:HEAD]
                nc.vector.tensor_copy(dst, st8["ps"].rearrange(
                    "p (h d) -> p h d", h=HPC))
            return [(460, mm(0, 4)), (460, mm(4, 4)), (0, evac)]

        def make_o_pset(qi, m, eng, pool=None, ev=None):
            sl = bass.ds(CH * qi, CH)
            st8 = {}
            pool = pool or psP

            def mm():
                if pool is stP:
                    t = pool.tile([P, 2 * CH], DT, tag="stP", name=f"pso{qi}{m}")
                    st8["ps"] = t[:, 0:CH]
                else:
                    st8["ps"] = pool.tile([P, CH], DT, tag="psP",
                                          name=f"pso{qi}{m}")
                for c in range(2):
                    for k in range(2):
                        nc.tensor.matmul(
                            st8["ps"][:, bass.ds(256 * c, 256)],
                            lhsT=wobig[:, bass.ds(EMBED * k + P * m, P)],
                            rhs=ohat_t[k][:, bass.ds(CH * qi + 256 * c, 256)],
                            start=(k == 0), stop=(k == 1))

            def evac():
                ysb = ypool.tile([P, CH], DTH, tag="ysb", name=f"ysb{qi}{m}")
                if ev is nc.scalar:
                    nc.scalar.copy(ysb[:], st8["ps"])
                else:
                    nc.vector.tensor_copy(ysb[:], st8["ps"])
                eng.dma_start(
                    out=yT.rearrange("(m p) s -> p m s", p=P)[:, m, sl],
                    in_=ysb[:])
            return [(470, mm), (0, evac)]

        def queue_qkv_chunk(qi):
            for f in range(4):
                chunk_q.extend(make_qk_pset(qi, f))
            for s in range(4):
                chunk_q.extend(make_v_pset(qi, s))

        def queue_oproj(qi, pool=None):
            for m in range(8):
                if pool is stP:
                    # tail: keep the output DMAs on the fast hardware DGEs
                    eng = nc.sync if m % 2 == 0 else nc.scalar
                    ev = nc.scalar if m % 2 == 1 else None
                else:
                    eng = nc.sync if m % 2 == 0 else nc.gpsimd
                    ev = None
                oproj_q.extend(make_o_pset(qi, m, eng, pool=pool, ev=ev))

        def fill(budget):
            while budget > 0 and (chunk_q or oproj_q):
                q = chunk_q if chunk_q else oproj_q
                cost, fn = q.popleft()
                fn()
                budget -= max(cost, 120)

        def flush(q):
            while q:
                _, fn = q.popleft()
                fn()

        # ---- attention pair: pipelined window loop ----
        LAG = 2

        def attn_pair(hp, qi):
            nk = 4 * qi + 4
            po = [psO.tile([P, CH], DT, tag="psO", name=f"po{hp}{qi}{hh}")
                  for hh in range(2)]
            sts = {}
            for w in range(nk + LAG):
                ki = w
                if ki < nk:
                    kr = ki - 4 * qi
                    trim = P * kr if kr >= 0 else 0
                    W = CH - trim
                    pst = stP.tile([P, 2 * CH], DT, tag="stP", name="pst")
                    for hh in range(2):
                        r0 = HEAD * hh
                        nc.tensor.matmul(
                            pst[:, bass.ds(CH * hh + trim, W)],
                            lhsT=kt_t[hp][r0:r0 + HEAD, bass.ds(P * ki, P)],
                            rhs=qt_t[hp][r0:r0 + HEAD, bass.ds(CH * qi + trim, W)],
                            start=True, stop=True)
                    st = stpool.tile([P, 2 * CH], DTB, tag="st", name="st")
                    psrc = pst.rearrange("p (h c) -> p h c", h=2)[:, :, trim:CH]
                    sdst = st.rearrange("p (h c) -> p h c", h=2)[:, :, trim:CH]
                    nc.scalar.activation(sdst, psrc, EXP, scale=0.125)
                    if kr >= 0:
                        for hh in range(2):
                            nc.vector.tensor_mul(
                                st[:, bass.ds(CH * hh + trim, W)],
                                st[:, bass.ds(CH * hh + trim, W)],
                                mask_sb[:, 0:W])
                    sts[ki] = (st, trim)
                    act_ns = (2 * W + 352) * 0.833
                else:
                    act_ns = 1200
                # AV lags QK by LAG windows; the first AV slips one extra
                # window (caught up at w=LAG+1) so the previous pair's
                # normalize has freed the psO ring before AV[0] queues.
                if w in (LAG, LAG + 1):
                    kavs = []
                elif w == LAG + 2:
                    kavs = [0, 1, 2][:nk]
                else:
                    kavs = [w - LAG]
                av_ns = 0
                for kav in kavs:
                    if not (0 <= kav < nk):
                        continue
                    st, trim = sts.pop(kav)
                    W = CH - trim
                    for hh in range(2):
                        h = 2 * hp + hh
                        nc.tensor.matmul(
                            po[hh][0:HEAD + 1, trim:CH],
                            lhsT=vtbig[:, bass.ds(VW * kav + (HEAD + 1) * h,
                                                  HEAD + 1)],
                            rhs=st[:, bass.ds(CH * hh + trim, W)],
                            start=(kav == 0), stop=(kav == nk - 1))
                    av_ns = 2 * (W * 0.42 + 40)
                qk_ns = (CH - max(0, ki - 4 * qi) * P) * 0.42 + 60 if ki < nk else 0
                fill(act_ns - qk_ns - av_ns + 150)
            # normalize: recip of the ones-row denominator, broadcast, scale.
            # On the last pair the outproj tail waits on this chain: copy the
            # denominators on the (idle) scalar engine and process in column
            # halves so the second half's chain is all that trails the last AV.
            last = (hp == 1 and qi == NCH - 1)
            if not last:
                for hh in range(2):
                    r0 = HEAD * hh
                    den = rpool.tile([1, CH], DT, tag=f"den{hh}", name="den")
                    nc.vector.tensor_copy(den[:], po[hh][HEAD:HEAD + 1, :])
                    recip = rpool.tile([1, CH], DT, tag="recip", name="recip")
                    nc.vector.reciprocal_approx_fast(recip[:], den[:])
                    recipb = rpool.tile([HEAD, CH], DT, tag="recipb", name="recipb")
                    nc.gpsimd.partition_broadcast(recipb[:], recip[:])
                    nc.vector.tensor_mul(
                        ohat_t[hp][r0:r0 + HEAD, bass.ds(CH * qi, CH)],
                        po[hh][0:HEAD, :], recipb[:])
            else:
                # tail chain: fresh tiles (no ring-reuse waits), scalar copies,
                # heads pipelined across scalar/DVE/gpsimd
                dens, recips = [], []
                for hh in range(2):
                    den = tailpool.tile([1, CH], DT, tag=f"tden{hh}", name="tden")
                    nc.scalar.copy(den[:], po[hh][HEAD:HEAD + 1, :])
                    dens.append(den)
                for hh in range(2):
                    recip = tailpool.tile([1, CH], DT, tag=f"trec{hh}", name="trec")
                    nc.vector.reciprocal_approx_fast(recip[:], dens[hh][:])
                    recips.append(recip)
                for hh in range(2):
                    r0 = HEAD * hh
                    recipb = tailpool.tile([HEAD, CH], DT, tag=f"trb{hh}", name="trb")
                    nc.gpsimd.partition_broadcast(recipb[:], recips[hh][:])
                    nc.vector.tensor_mul(
                        ohat_t[hp][r0:r0 + HEAD, bass.ds(CH * qi, CH)],
                        po[hh][0:HEAD, :], recipb[:])

        # ---- schedule ----
        queue_qkv_chunk(0)
        flush(chunk_q)
        queue_qkv_chunk(1)
        attn_pair(0, 0)
        attn_pair(1, 0)
        flush(chunk_q)
        queue_qkv_chunk(2)
        attn_pair(0, 1)
        attn_pair(1, 1)
        flush(chunk_q)
        queue_qkv_chunk(3)
        queue_oproj(0)
        attn_pair(0, 2)
        attn_pair(1, 2)
        flush(chunk_q)
        queue_oproj(1)
        queue_oproj(2)
        attn_pair(0, 3)
        attn_pair(1, 3)
        flush(oproj_q)
        queue_oproj(3, pool=stP)
        flush(oproj_q)

    nc.compile()
    return nc


def _make_mask():
    p = np.arange(P)[:, None]
    j = np.arange(CH)[None, :]
    return (p <= j).astype(np.float32)


def kernel(x, W_qkv, b_qkv, W_out, b_out):
    global LAST_EXEC_NS, LAST_RESULTS
    x = np.asarray(x, dtype=np.float32)
    W_qkv = np.asarray(W_qkv, dtype=np.float32)
    b_qkv = np.asarray(b_qkv, dtype=np.float32)
    W_out = np.asarray(W_out, dtype=np.float32)
    b_out = np.asarray(b_out, dtype=np.float32)

    nc = _build_program()
    mask = _make_mask()

    in_maps = []
    for c in range(N_CORES):
        b, g = divmod(c, HPC)
        q0 = GCOLS * g
        wq = W_qkv[q0:q0 + GCOLS]                    # [256, 1024]
        wk = W_qkv[EMBED + q0:EMBED + q0 + GCOLS]
        wv = W_qkv[2 * EMBED + q0:2 * EMBED + q0 + GCOLS]
        bq = b_qkv[q0:q0 + GCOLS]
        bk = b_qkv[EMBED + q0:EMBED + q0 + GCOLS]
        bqk = np.stack([bq[0:P], bq[P:2 * P], bk[0:P], bk[P:2 * P]],
                       axis=1).astype(np.float32)   # [128, 4]
        wqkt = np.concatenate([wq, wk], 0).T            # [1024 embed, 512]
        wqkf = wqkt.reshape(8, P, 4, P).transpose(1, 2, 0, 3).reshape(P, 4 * 8 * P)
        # xP[p, qi, i, s]: chunk-major so each chunk is one contiguous DMA
        xp = x[b].T.reshape(8, P, NCH, CH).transpose(1, 2, 0, 3).reshape(P, 8 * SEQ)
        # wvF in wvbig layout [p, i, 256]
        wvf = wv.T.reshape(8, P, GCOLS).transpose(1, 0, 2).reshape(P, 8 * GCOLS)
        # woF in wobig layout [p, k, 1024]
        wof = W_out[:, q0:q0 + GCOLS].T.reshape(2, P, EMBED).transpose(
            1, 0, 2).reshape(P, 2 * EMBED)
        in_maps.append({
            "xP": np.ascontiguousarray(xp).astype(ml_dtypes.bfloat16),
            "wqkF": np.ascontiguousarray(wqkf).astype(ml_dtypes.bfloat16),
            "wvF": np.ascontiguousarray(wvf).astype(ml_dtypes.bfloat16),
            "bqk": np.ascontiguousarray(bqk),
            "woF": np.ascontiguousarray(wof).astype(ml_dtypes.bfloat16),
            "maskT": mask.astype(ml_dtypes.bfloat16),
        })

    want_trace = bool(int(os.environ.get("KTRACE", "0")))
    if want_trace:
        try:
            import antenv.axon_hooks  # noqa: F401
        except ImportError:
            want_trace = False
    res = run_bass_kernel_spmd(nc, in_maps, list(range(N_CORES)),
                               trace=want_trace,
                               tmpdir=os.environ.get("KTRACE_DIR") or None)
    LAST_EXEC_NS = res.exec_time_ns
    LAST_RESULTS = res

    out = np.empty((BATCH, SEQ, EMBED), dtype=np.float32)
    crow = (b_out + W_out @ b_qkv[2 * EMBED:]).astype(np.float32)
    for b in range(BATCH):
        acc = np.zeros((EMBED, SEQ), dtype=np.float32)
        for g in range(HPC):
            acc += res.results[HPC * b + g]["yT"].astype(np.float32)
        out[b] = acc.T + crow[None, :]
    return out


# revision 6
# speedup vs baseline: 1.0258x; 1.0030x over previous
"""Multi-head self-attention (batch=2, seq=2048, embed=1024, heads=16, causal)
sharded over 8 NeuronCores: data-parallel over batch (x2) and tensor-parallel
over heads (x4 groups of 4 heads).

Per core: qkvT projection with the bias folded into the psum evacuation,
row-tiled QK^T (the two 64-contraction head matmuls run concurrently on the
PE), causal softmax with the denominator folded into the AV matmul via a
ones-column on V, and a partial output projection; the host sums the four
partials per batch and adds the constant row b_out + W_out @ b_v.

Schedule: software-pipelined attention windows (AV lags QK by 2 windows so
the Scalar-engine exp latency never head-of-line-blocks the in-order PE
queue), diagonal-block column trimming (QK/exp/mask/AV restricted to the
causally-live column range), projection matmuls interleaved into the
attention windows as PE fill with double-buffered psum psets, all of x
resident in SBUF, inputs host-packed to make every DMA contiguous on the
hardware DGE queues (gpsimd's software DGE only carries tiny/late
transfers), and PE warmup matmuls burning the HAM cold window during the
startup DMAs.
"""

import os
from collections import deque

import ml_dtypes
import numpy as np
from contextlib import ExitStack

import concourse.bass as bass
import concourse.mybir as mybir
import concourse.tile as tile
from concourse import bacc
from concourse.bass_utils import run_bass_kernel_spmd

N_HEADS = 16
EMBED = 1024
HEAD = 64
SEQ = 2048
BATCH = 2
N_CORES = 8
HPC = 4                # heads per core
GCOLS = HPC * HEAD     # 256 embed columns per head group
P = 128
CH = 512               # seq chunk
NCH = SEQ // CH        # 4
KT = SEQ // P          # 16 k tiles
VW = HPC * (HEAD + 1)  # v row width per ktile (ones column at 64 per head)

DT = mybir.dt.float32
DTB = mybir.dt.bfloat16
DTH = mybir.dt.float16

LAST_EXEC_NS = None
LAST_RESULTS = None


def _build_program():
    nc = bacc.Bacc("TRN2", target_bir_lowering=False, debug=False,
                   num_devices=N_CORES)
    # All inputs host-packed into the exact SBUF-resident layouts so every
    # DMA is a contiguous per-partition run (hardware DGE at full rate).
    # xP[p, 4096*qi + 512*i + s]: x chunk-major; wqkF[p, 1024*f + 128*i + c];
    # wvF = wvbig layout; woF = wobig layout.
    xP = nc.dram_tensor("xP", [P, 8 * SEQ], DTB, kind="ExternalInput")
    wqkF = nc.dram_tensor("wqkF", [P, 4 * 8 * P], DTB, kind="ExternalInput")
    wvF = nc.dram_tensor("wvF", [P, 8 * GCOLS], DTB, kind="ExternalInput")
    bqk = nc.dram_tensor("bqk", [P, 4], DT, kind="ExternalInput")
    woF = nc.dram_tensor("woF", [P, 2 * EMBED], DTB, kind="ExternalInput")
    maskT = nc.dram_tensor("maskT", [P, CH], DTB, kind="ExternalInput")
    yT = nc.dram_tensor("yT", [EMBED, SEQ], DTH, kind="ExternalOutput")

    EXP = mybir.ActivationFunctionType.Exp

    with tile.TileContext(nc) as tc, ExitStack() as ctx:
        const = ctx.enter_context(tc.tile_pool(name="const", bufs=1))
        stpool = ctx.enter_context(tc.tile_pool(name="stpool", bufs=6))
        rpool = ctx.enter_context(tc.tile_pool(name="rpool", bufs=4))
        tailpool = ctx.enter_context(tc.tile_pool(name="tailpool", bufs=1))
        ypool = ctx.enter_context(tc.tile_pool(name="ypool", bufs=4))
        stP = ctx.enter_context(tc.tile_pool(name="stP", bufs=2, space="PSUM"))
        psO = ctx.enter_context(tc.tile_pool(name="psO", bufs=2, space="PSUM"))
        psP = ctx.enter_context(tc.tile_pool(name="psP", bufs=2, space="PSUM"))

        # ---- persistent SBUF residents ----
        wqkbig = const.tile([P, 8 * 2 * GCOLS], DTB, tag="wqkbig")
        wvbig = const.tile([P, 8 * GCOLS], DTB, tag="wvbig")
        wobig = const.tile([P, 2 * EMBED], DTB, tag="wobig")
        x_all = const.tile([P, 8 * SEQ], DTB, tag="xall")
        qt_t = [const.tile([P, SEQ], DTB, tag=f"qt{a}", name=f"qt{a}") for a in range(2)]
        kt_t = [const.tile([P, SEQ], DTB, tag=f"kt{a}", name=f"kt{a}") for a in range(2)]
        vtbig = const.tile([P, KT * VW], DTB, tag="vtbig")
        ohat_t = [const.tile([P, SEQ], DTB, tag=f"ohat{a}", name=f"ohat{a}") for a in range(2)]
        bqk_sb = const.tile([P, 4], DT, tag="bqk")
        mask_sb = const.tile([P, CH], DTB, tag="mask")

        # ---- PE warmup: burn the HAM cold window during the DMA wait.
        # Feed from a memset tile so the warmup is not gated on any DMA.
        wsrc = const.tile([P, HEAD], DTB, tag="wsrc")
        nc.gpsimd.memset(wsrc[:], 0.25)
        NWARM = 80
        wps = psP.tile([P, HEAD], DT, tag="psP", name="warm")
        for i in range(NWARM):
            nc.tensor.matmul(wps[0:HEAD, :], lhsT=wsrc[:, 0:HEAD],
                             rhs=wsrc[:, 0:HEAD],
                             start=(i == 0), stop=(i == NWARM - 1))
        wsc = rpool.tile([HEAD, HEAD], DT, tag="warmev", name="warmev")
        nc.vector.tensor_copy(wsc[:], wps[0:HEAD, :])

        # ---- preamble: gpsimd DMAs go through the slow software DGE
        # (~50 GB/s), so only tiny or late-needed transfers ride there; the
        # two hardware DGE queues (sync, scalar) carry everything that gates
        # the first qkv psets, in need-order.
        nc.gpsimd.dma_start(out=mask_sb, in_=maskT[:])
        nc.gpsimd.dma_start(out=bqk_sb, in_=bqk[:])
        nc.gpsimd.memset(
            vtbig.rearrange("p (t h d) -> p t h d",
                            t=KT, h=HPC)[:, :, :, HEAD:HEAD + 1], 1.0)

        def wqk_f(f):
            return bass.ds(8 * P * f, 8 * P)

        def xch(qi, half):
            return bass.ds(8 * CH * qi + 4 * CH * half, 4 * CH)

        def xq(qi, quarter):
            return bass.ds(8 * CH * qi + 2 * CH * quarter, 2 * CH)

        nc.sync.dma_start(out=wqkbig[:, bass.ds(0, 4 * P)],
                          in_=wqkF[:, bass.ds(0, 4 * P)])
        nc.scalar.dma_start(out=wqkbig[:, bass.ds(4 * P, 4 * P)],
                            in_=wqkF[:, bass.ds(4 * P, 4 * P)])
        nc.scalar.dma_start(out=x_all[:, xq(0, 0)], in_=xP[:, xq(0, 0)])
        nc.sync.dma_start(out=x_all[:, xq(0, 1)], in_=xP[:, xq(0, 1)])
        nc.scalar.dma_start(out=x_all[:, xq(0, 2)], in_=xP[:, xq(0, 2)])
        nc.sync.dma_start(out=x_all[:, xq(0, 3)], in_=xP[:, xq(0, 3)])
        nc.scalar.dma_start(out=wqkbig[:, wqk_f(1)], in_=wqkF[:, wqk_f(1)])
        nc.sync.dma_start(out=wqkbig[:, wqk_f(2)], in_=wqkF[:, wqk_f(2)])
        nc.scalar.dma_start(out=wqkbig[:, wqk_f(3)], in_=wqkF[:, wqk_f(3)])
        nc.sync.dma_start(out=wvbig[:, 0:4 * GCOLS], in_=wvF[:, 0:4 * GCOLS])
        nc.scalar.dma_start(out=wvbig[:, 4 * GCOLS:], in_=wvF[:, 4 * GCOLS:])
        for qi in range(1, NCH):
            nc.sync.dma_start(out=x_all[:, xch(qi, 0)], in_=xP[:, xch(qi, 0)])
            nc.scalar.dma_start(out=x_all[:, xch(qi, 1)], in_=xP[:, xch(qi, 1)])
        nc.gpsimd.dma_start(out=wobig[:], in_=woF[:])

        # ---- projection fill units ----
        # Each unit: (pe_ns_estimate, closure). Closures share per-pset state
        # so the accumulation psum tile is created at the first sub-unit.
        chunk_q = deque()   # qkv-chunk units: must complete before their pair
        oproj_q = deque()   # out-projection units: opportunistic

        def make_qk_pset(qi, f):
            sl = bass.ds(CH * qi, CH)
            st8 = {}

            def mm(c, i0, n):
                # N=256 half-column chains pipeline at ~120ns/MM where the
                # N=512 chains pay a ~63ns/MM bubble
                def go():
                    if "ps" not in st8:
                        st8["ps"] = psP.tile([P, CH], DT, tag="psP",
                                             name=f"psqk{qi}{f}")
                    ps = st8["ps"]
                    for i in range(i0, i0 + n):
                        nc.tensor.matmul(
                            ps[:, bass.ds(256 * c, 256)],
                            lhsT=wqkbig[:, bass.ds(8 * P * f + P * i, P)],
                            rhs=x_all[:, bass.ds(8 * CH * qi + CH * i + 256 * c,
                                                 256)],
                            start=(i == 0), stop=(i == 7))
                return go

            def evac():
                dst = (qt_t if f < 2 else kt_t)[f % 2]
                nc.vector.tensor_scalar_add(dst[:, sl], st8["ps"],
                                            bqk_sb[:, f:f + 1])
            return [(520, mm(0, 0, 4)), (520, mm(0, 4, 4)),
                    (520, mm(1, 0, 4)), (520, mm(1, 4, 4)), (0, evac)]

        def make_v_pset(qi, s):
            ti = 4 * qi + s
            st8 = {}

            def mm(i0, n):
                def go():
                    if "ps" not in st8:
                        st8["ps"] = psP.tile([P, GCOLS], DT, tag="psP",
                                             name=f"psv{qi}{s}")
                    ps = st8["ps"]
                    for i in range(i0, i0 + n):
                        nc.tensor.matmul(
                            ps,
                            lhsT=x_all[:, bass.ds(8 * CH * qi + CH * i + P * s, P)],
                            rhs=wvbig[:, bass.ds(GCOLS * i, GCOLS)],
                            start=(i == 0), stop=(i == 7))
                return go

            def evac():
                dst = vtbig[:, bass.ds(VW * ti, VW)].rearrange(
                    "p (h d) -> p h d", h=HPC)[:, :, 0:HEAD]
                nc.vector.tensor_copy(dst, st8["ps"].rearrange(
                    "p (h d) -> p h d", h=HPC))
            return [(460, mm(0, 4)), (460, mm(4, 4)), (0, evac)]

        def make_o_pset(qi, m, eng, pool=None, ev=None, ysb_pair=None):
            sl = bass.ds(CH * qi, CH)
            st8 = {}
            pool = pool or psP

            def mm():
                if pool is stP:
                    t = pool.tile([P, 2 * CH], DT, tag="stP", name=f"pso{qi}{m}")
                    st8["ps"] = t[:, 0:CH]
                else:
                    st8["ps"] = pool.tile([P, CH], DT, tag="psP",
                                          name=f"pso{qi}{m}")
                for c in range(2):
                    for k in range(2):
                        nc.tensor.matmul(
                            st8["ps"][:, bass.ds(256 * c, 256)],
                            lhsT=wobig[:, bass.ds(EMBED * k + P * m, P)],
                            rhs=ohat_t[k][:, bass.ds(CH * qi + 256 * c, 256)],
                            start=(k == 0), stop=(k == 1))

            def evac():
                # m-pairs share one staging tile; the odd member DMAs both
                # halves in a single transfer (half the DMAs and semaphores)
                if m % 2 == 0:
                    ysb_pair["t"] = ypool.tile([P, 2 * CH], DTH, tag="ysb",
                                               name=f"ysb{qi}{m}")
                ysb = ysb_pair["t"]
                half = ysb[:, bass.ds(CH * (m % 2), CH)]
                if ev is nc.scalar:
                    nc.scalar.copy(half, st8["ps"])
                else:
                    nc.vector.tensor_copy(half, st8["ps"])
                if m % 2 == 1:
                    eng.dma_start(
                        out=yT.rearrange("(j q p) s -> p j q s",
                                         q=2, p=P)[:, m // 2, :, sl],
                        in_=ysb.rearrange("p (q s) -> p q s", q=2))
            return [(470, mm), (0, evac)]

        def queue_qkv_chunk(qi):
            for f in range(4):
                chunk_q.extend(make_qk_pset(qi, f))
            for s in range(4):
                chunk_q.extend(make_v_pset(qi, s))

        def queue_oproj(qi, pool=None):
            ysb_pair = {}
            for m in range(8):
                if pool is stP:
                    # tail: keep the output DMAs on the fast hardware DGEs
                    eng = nc.sync if m % 2 == 0 else nc.scalar
                    ev = nc.scalar if m % 2 == 1 else None
                else:
                    eng = nc.sync if m % 2 == 0 else nc.gpsimd
                    ev = None
                oproj_q.extend(make_o_pset(qi, m, eng, pool=pool, ev=ev,
                                           ysb_pair=ysb_pair))

        def fill(budget):
            while budget > 0 and (chunk_q or oproj_q):
                q = chunk_q if chunk_q else oproj_q
                cost, fn = q.popleft()
                fn()
                budget -= max(cost, 120)

        def flush(q):
            while q:
                _, fn = q.popleft()
                fn()

        # ---- attention pair: pipelined window loop ----
        LAG = 2

        def attn_pair(hp, qi):
            nk = 4 * qi + 4
            po = [psO.tile([P, CH], DT, tag="psO", name=f"po{hp}{qi}{hh}")
                  for hh in range(2)]
            sts = {}
            for w in range(nk + LAG):
                ki = w
                if ki < nk:
                    kr = ki - 4 * qi
                    trim = P * kr if kr >= 0 else 0
                    W = CH - trim
                    pst = stP.tile([P, 2 * CH], DT, tag="stP", name="pst")
                    for hh in range(2):
                        r0 = HEAD * hh
                        nc.tensor.matmul(
                            pst[:, bass.ds(CH * hh + trim, W)],
                            lhsT=kt_t[hp][r0:r0 + HEAD, bass.ds(P * ki, P)],
                            rhs=qt_t[hp][r0:r0 + HEAD, bass.ds(CH * qi + trim, W)],
                            start=True, stop=True)
                    st = stpool.tile([P, 2 * CH], DTB, tag="st", name="st")
                    psrc = pst.rearrange("p (h c) -> p h c", h=2)[:, :, trim:CH]
                    sdst = st.rearrange("p (h c) -> p h c", h=2)[:, :, trim:CH]
                    nc.scalar.activation(sdst, psrc, EXP, scale=0.125)
                    if kr >= 0:
                        for hh in range(2):
                            nc.vector.tensor_mul(
                                st[:, bass.ds(CH * hh + trim, W)],
                                st[:, bass.ds(CH * hh + trim, W)],
                                mask_sb[:, 0:W])
                    sts[ki] = (st, trim)
                    act_ns = (2 * W + 352) * 0.833
                else:
                    act_ns = 1200
                # AV lags QK by LAG windows; the first AV slips one extra
                # window (caught up at w=LAG+1) so the previous pair's
                # normalize has freed the psO ring before AV[0] queues.
                if w in (LAG, LAG + 1):
                    kavs = []
                elif w == LAG + 2:
                    kavs = [0, 1, 2][:nk]
                else:
                    kavs = [w - LAG]
                av_ns = 0
                for kav in kavs:
                    if not (0 <= kav < nk):
                        continue
                    st, trim = sts.pop(kav)
                    W = CH - trim
                    for hh in range(2):
                        h = 2 * hp + hh
                        nc.tensor.matmul(
                            po[hh][0:HEAD + 1, trim:CH],
                            lhsT=vtbig[:, bass.ds(VW * kav + (HEAD + 1) * h,
                                                  HEAD + 1)],
                            rhs=st[:, bass.ds(CH * hh + trim, W)],
                            start=(kav == 0), stop=(kav == nk - 1))
                    av_ns = 2 * (W * 0.42 + 40)
                qk_ns = (CH - max(0, ki - 4 * qi) * P) * 0.42 + 60 if ki < nk else 0
                fill(act_ns - qk_ns - av_ns + 150)
            # normalize: recip of the ones-row denominator, broadcast, scale.
            # On the last pair the outproj tail waits on this chain: copy the
            # denominators on the (idle) scalar engine and process in column
            # halves so the second half's chain is all that trails the last AV.
            last = (hp == 1 and qi == NCH - 1)
            if not last:
                for hh in range(2):
                    r0 = HEAD * hh
                    den = rpool.tile([1, CH], DT, tag=f"den{hh}", name="den")
                    nc.vector.tensor_copy(den[:], po[hh][HEAD:HEAD + 1, :])
                    recip = rpool.tile([1, CH], DT, tag="recip", name="recip")
                    nc.vector.reciprocal_approx_fast(recip[:], den[:])
                    recipb = rpool.tile([HEAD, CH], DT, tag="recipb", name="recipb")
                    nc.gpsimd.partition_broadcast(recipb[:], recip[:])
                    nc.vector.tensor_mul(
                        ohat_t[hp][r0:r0 + HEAD, bass.ds(CH * qi, CH)],
                        po[hh][0:HEAD, :], recipb[:])
            else:
                # tail chain: fresh tiles (no ring-reuse waits), scalar copies,
                # heads pipelined across scalar/DVE/gpsimd
                dens, recips = [], []
                for hh in range(2):
                    den = tailpool.tile([1, CH], DT, tag=f"tden{hh}", name="tden")
                    nc.scalar.copy(den[:], po[hh][HEAD:HEAD + 1, :])
                    dens.append(den)
                for hh in range(2):
                    recip = tailpool.tile([1, CH], DT, tag=f"trec{hh}", name="trec")
                    nc.vector.reciprocal_approx_fast(recip[:], dens[hh][:])
                    recips.append(recip)
                for hh in range(2):
                    r0 = HEAD * hh
                    recipb = tailpool.tile([HEAD, CH], DT, tag=f"trb{hh}", name="trb")
                    nc.gpsimd.partition_broadcast(recipb[:], recips[hh][:])
                    nc.vector.tensor_mul(
                        ohat_t[hp][r0:r0 + HEAD, bass.ds(CH * qi, CH)],
                        po[hh][0:HEAD, :], recipb[:])

        # ---- schedule ----
        queue_qkv_chunk(0)
        flush(chunk_q)
        queue_qkv_chunk(1)
        attn_pair(0, 0)
        attn_pair(1, 0)
        flush(chunk_q)
        queue_qkv_chunk(2)
        attn_pair(0, 1)
        attn_pair(1, 1)
        flush(chunk_q)
        queue_qkv_chunk(3)
        queue_oproj(0)
        attn_pair(0, 2)
        attn_pair(1, 2)
        flush(chunk_q)
        queue_oproj(1)
        queue_oproj(2)
        attn_pair(0, 3)
        attn_pair(1, 3)
        flush(oproj_q)
        queue_oproj(3, pool=stP)
        flush(oproj_q)

    nc.compile()
    return nc


def _make_mask():
    p = np.arange(P)[:, None]
    j = np.arange(CH)[None, :]
    return (p <= j).astype(np.float32)


def kernel(x, W_qkv, b_qkv, W_out, b_out):
    global LAST_EXEC_NS, LAST_RESULTS
    x = np.asarray(x, dtype=np.float32)
    W_qkv = np.asarray(W_qkv, dtype=np.float32)
    b_qkv = np.asarray(b_qkv, dtype=np.float32)
    W_out = np.asarray(W_out, dtype=np.float32)
    b_out = np.asarray(b_out, dtype=np.float32)

    nc = _build_program()
    mask = _make_mask()

    in_maps = []
    for c in range(N_CORES):
        b, g = divmod(c, HPC)
        q0 = GCOLS * g
        wq = W_qkv[q0:q0 + GCOLS]                    # [256, 1024]
        wk = W_qkv[EMBED + q0:EMBED + q0 + GCOLS]
        wv = W_qkv[2 * EMBED + q0:2 * EMBED + q0 + GCOLS]
        bq = b_qkv[q0:q0 + GCOLS]
        bk = b_qkv[EMBED + q0:EMBED + q0 + GCOLS]
        bqk = np.stack([bq[0:P], bq[P:2 * P], bk[0:P], bk[P:2 * P]],
                       axis=1).astype(np.float32)   # [128, 4]
        wqkt = np.concatenate([wq, wk], 0).T            # [1024 embed, 512]
        wqkf = wqkt.reshape(8, P, 4, P).transpose(1, 2, 0, 3).reshape(P, 4 * 8 * P)
        # xP[p, qi, i, s]: chunk-major so each chunk is one contiguous DMA
        xp = x[b].T.reshape(8, P, NCH, CH).transpose(1, 2, 0, 3).reshape(P, 8 * SEQ)
        # wvF in wvbig layout [p, i, 256]
        wvf = wv.T.reshape(8, P, GCOLS).transpose(1, 0, 2).reshape(P, 8 * GCOLS)
        # woF in wobig layout [p, k, 1024]
        wof = W_out[:, q0:q0 + GCOLS].T.reshape(2, P, EMBED).transpose(
            1, 0, 2).reshape(P, 2 * EMBED)
        in_maps.append({
            "xP": np.ascontiguousarray(xp).astype(ml_dtypes.bfloat16),
            "wqkF": np.ascontiguousarray(wqkf).astype(ml_dtypes.bfloat16),
            "wvF": np.ascontiguousarray(wvf).astype(ml_dtypes.bfloat16),
            "bqk": np.ascontiguousarray(bqk),
            "woF": np.ascontiguousarray(wof).astype(ml_dtypes.bfloat16),
            "maskT": mask.astype(ml_dtypes.bfloat16),
        })

    want_trace = bool(int(os.environ.get("KTRACE", "0")))
    if want_trace:
        try:
            import antenv.axon_hooks  # noqa: F401
        except ImportError:
            want_trace = False
    res = run_bass_kernel_spmd(nc, in_maps, list(range(N_CORES)),
                               trace=want_trace,
                               tmpdir=os.environ.get("KTRACE_DIR") or None)
    LAST_EXEC_NS = res.exec_time_ns
    LAST_RESULTS = res

    out = np.empty((BATCH, SEQ, EMBED), dtype=np.float32)
    crow = (b_out + W_out @ b_qkv[2 * EMBED:]).astype(np.float32)
    for b in range(BATCH):
        acc = np.zeros((EMBED, SEQ), dtype=np.float32)
        for g in range(HPC):
            acc += res.results[HPC * b + g]["yT"].astype(np.float32)
        out[b] = acc.T + crow[None, :]
    return out


# revision 8
# speedup vs baseline: 1.0307x; 1.0048x over previous
"""Multi-head self-attention (batch=2, seq=2048, embed=1024, heads=16, causal)
sharded over 8 NeuronCores: data-parallel over batch (x2) and tensor-parallel
over heads (x4 groups of 4 heads).

Per core: qkvT projection with the bias folded into the psum evacuation,
row-tiled QK^T (the two 64-contraction head matmuls run concurrently on the
PE), causal softmax with the denominator folded into the AV matmul via a
ones-column on V, and a partial output projection; the host sums the four
partials per batch and adds the constant row b_out + W_out @ b_v.

Schedule: software-pipelined attention windows (AV lags QK by 2 windows so
the Scalar-engine exp latency never head-of-line-blocks the in-order PE
queue), diagonal-block column trimming (QK/exp/mask/AV restricted to the
causally-live column range), projection matmuls interleaved into the
attention windows as PE fill with double-buffered psum psets, all of x
resident in SBUF, inputs host-packed to make every DMA contiguous on the
hardware DGE queues (gpsimd's software DGE only carries tiny/late
transfers), and PE warmup matmuls burning the HAM cold window during the
startup DMAs.
"""

import os
from collections import deque

import ml_dtypes
import numpy as np
from contextlib import ExitStack

import concourse.bass as bass
import concourse.mybir as mybir
import concourse.tile as tile
from concourse import bacc
from concourse.bass_utils import run_bass_kernel_spmd

N_HEADS = 16
EMBED = 1024
HEAD = 64
SEQ = 2048
BATCH = 2
N_CORES = 8
HPC = 4                # heads per core
GCOLS = HPC * HEAD     # 256 embed columns per head group
P = 128
CH = 512               # seq chunk
NCH = SEQ // CH        # 4
KT = SEQ // P          # 16 k tiles
VW = HPC * (HEAD + 1)  # v row width per ktile (ones column at 64 per head)

DT = mybir.dt.float32
DTB = mybir.dt.bfloat16
DTH = mybir.dt.float16

LAST_EXEC_NS = None
LAST_RESULTS = None


def _build_program():
    nc = bacc.Bacc("TRN2", target_bir_lowering=False, debug=False,
                   num_devices=N_CORES)
    # All inputs host-packed into the exact SBUF-resident layouts so every
    # DMA is a contiguous per-partition run (hardware DGE at full rate).
    # xP[p, 4096*qi + 512*i + s]: x chunk-major; wqkF[p, 1024*f + 128*i + c];
    # wvF = wvbig layout; woF = wobig layout.
    xP = nc.dram_tensor("xP", [P, 8 * SEQ], DTB, kind="ExternalInput")
    wqkF = nc.dram_tensor("wqkF", [P, 4 * 8 * P], DTB, kind="ExternalInput")
    wvF = nc.dram_tensor("wvF", [P, 8 * GCOLS], DTB, kind="ExternalInput")
    bqk = nc.dram_tensor("bqk", [P, 4], DT, kind="ExternalInput")
    woF = nc.dram_tensor("woF", [P, 2 * EMBED], DTB, kind="ExternalInput")
    maskT = nc.dram_tensor("maskT", [P, CH], DTB, kind="ExternalInput")
    yT = nc.dram_tensor("yT", [EMBED, SEQ], DTH, kind="ExternalOutput")

    EXP = mybir.ActivationFunctionType.Exp

    with tile.TileContext(nc) as tc, ExitStack() as ctx:
        const = ctx.enter_context(tc.tile_pool(name="const", bufs=1))
        stpool = ctx.enter_context(tc.tile_pool(name="stpool", bufs=6))
        rpool = ctx.enter_context(tc.tile_pool(name="rpool", bufs=4))
        tailpool = ctx.enter_context(tc.tile_pool(name="tailpool", bufs=1))
        ypool = ctx.enter_context(tc.tile_pool(name="ypool", bufs=4))
        stP = ctx.enter_context(tc.tile_pool(name="stP", bufs=2, space="PSUM"))
        psO = ctx.enter_context(tc.tile_pool(name="psO", bufs=2, space="PSUM"))
        psP = ctx.enter_context(tc.tile_pool(name="psP", bufs=2, space="PSUM"))

        # ---- persistent SBUF residents ----
        wqkbig = const.tile([P, 8 * 2 * GCOLS], DTB, tag="wqkbig")
        wvbig = const.tile([P, 8 * GCOLS], DTB, tag="wvbig")
        wobig = const.tile([P, 2 * EMBED], DTB, tag="wobig")
        x_all = const.tile([P, 8 * SEQ], DTB, tag="xall")
        qt_t = [const.tile([P, SEQ], DTB, tag=f"qt{a}", name=f"qt{a}") for a in range(2)]
        kt_t = [const.tile([P, SEQ], DTB, tag=f"kt{a}", name=f"kt{a}") for a in range(2)]
        vtbig = const.tile([P, KT * VW], DTB, tag="vtbig")
        ohat_t = [const.tile([P, SEQ], DTB, tag=f"ohat{a}", name=f"ohat{a}") for a in range(2)]
        bqk_sb = const.tile([P, 4], DT, tag="bqk")
        mask_sb = const.tile([P, CH], DTB, tag="mask")

        # ---- PE warmup: burn the HAM cold window during the DMA wait.
        # Feed from a memset tile so the warmup is not gated on any DMA.
        wsrc = const.tile([P, HEAD], DTB, tag="wsrc")
        nc.gpsimd.memset(wsrc[:], 0.25)
        NWARM = 80
        wps = psP.tile([P, HEAD], DT, tag="psP", name="warm")
        for i in range(NWARM):
            nc.tensor.matmul(wps[0:HEAD, :], lhsT=wsrc[:, 0:HEAD],
                             rhs=wsrc[:, 0:HEAD],
                             start=(i == 0), stop=(i == NWARM - 1))
        wsc = rpool.tile([HEAD, HEAD], DT, tag="warmev", name="warmev")
        nc.vector.tensor_copy(wsc[:], wps[0:HEAD, :])

        # ---- preamble: gpsimd DMAs go through the slow software DGE
        # (~50 GB/s), so only tiny or late-needed transfers ride there; the
        # two hardware DGE queues (sync, scalar) carry everything that gates
        # the first qkv psets, in need-order.
        nc.gpsimd.dma_start(out=mask_sb, in_=maskT[:])
        nc.gpsimd.dma_start(out=bqk_sb, in_=bqk[:])
        nc.gpsimd.memset(
            vtbig.rearrange("p (t h d) -> p t h d",
                            t=KT, h=HPC)[:, :, :, HEAD:HEAD + 1], 1.0)

        def wqk_f(f):
            return bass.ds(8 * P * f, 8 * P)

        def xch(qi, half):
            return bass.ds(8 * CH * qi + 4 * CH * half, 4 * CH)

        def xq(qi, quarter):
            return bass.ds(8 * CH * qi + 2 * CH * quarter, 2 * CH)

        nc.sync.dma_start(out=wqkbig[:, bass.ds(0, 4 * P)],
                          in_=wqkF[:, bass.ds(0, 4 * P)])
        nc.scalar.dma_start(out=wqkbig[:, bass.ds(4 * P, 4 * P)],
                            in_=wqkF[:, bass.ds(4 * P, 4 * P)])
        nc.scalar.dma_start(out=x_all[:, xq(0, 0)], in_=xP[:, xq(0, 0)])
        nc.sync.dma_start(out=x_all[:, xq(0, 1)], in_=xP[:, xq(0, 1)])
        nc.scalar.dma_start(out=x_all[:, xq(0, 2)], in_=xP[:, xq(0, 2)])
        nc.sync.dma_start(out=x_all[:, xq(0, 3)], in_=xP[:, xq(0, 3)])
        nc.scalar.dma_start(out=wqkbig[:, wqk_f(1)], in_=wqkF[:, wqk_f(1)])
        nc.sync.dma_start(out=wqkbig[:, wqk_f(2)], in_=wqkF[:, wqk_f(2)])
        nc.scalar.dma_start(out=wqkbig[:, wqk_f(3)], in_=wqkF[:, wqk_f(3)])
        nc.sync.dma_start(out=wvbig[:, 0:4 * GCOLS], in_=wvF[:, 0:4 * GCOLS])
        nc.scalar.dma_start(out=wvbig[:, 4 * GCOLS:], in_=wvF[:, 4 * GCOLS:])
        for qi in range(1, NCH):
            nc.sync.dma_start(out=x_all[:, xch(qi, 0)], in_=xP[:, xch(qi, 0)])
            nc.scalar.dma_start(out=x_all[:, xch(qi, 1)], in_=xP[:, xch(qi, 1)])
        nc.gpsimd.dma_start(out=wobig[:], in_=woF[:])

        # ---- projection fill units ----
        # Each unit: (pe_ns_estimate, closure). Closures share per-pset state
        # so the accumulation psum tile is created at the first sub-unit.
        chunk_q = deque()   # qkv-chunk units: must complete before their pair
        oproj_q = deque()   # out-projection units: opportunistic

        def make_qk_pset(qi, f):
            sl = bass.ds(CH * qi, CH)
            st8 = {}

            def mm(c, i0, n):
                # N=256 half-column chains pipeline at ~120ns/MM where the
                # N=512 chains pay a ~63ns/MM bubble
                def go():
                    if "ps" not in st8:
                        st8["ps"] = psP.tile([P, CH], DT, tag="psP",
                                             name=f"psqk{qi}{f}")
                    ps = st8["ps"]
                    for i in range(i0, i0 + n):
                        nc.tensor.matmul(
                            ps[:, bass.ds(256 * c, 256)],
                            lhsT=wqkbig[:, bass.ds(8 * P * f + P * i, P)],
                            rhs=x_all[:, bass.ds(8 * CH * qi + CH * i + 256 * c,
                                                 256)],
                            start=(i == 0), stop=(i == 7))
                return go

            def evac():
                dst = (qt_t if f < 2 else kt_t)[f % 2]
                nc.vector.tensor_scalar_add(dst[:, sl], st8["ps"],
                                            bqk_sb[:, f:f + 1])
            return [(520, mm(0, 0, 4)), (520, mm(0, 4, 4)),
                    (520, mm(1, 0, 4)), (520, mm(1, 4, 4)), (0, evac)]

        def make_v_pset(qi, s):
            ti = 4 * qi + s
            st8 = {}

            def mm(i0, n):
                def go():
                    if "ps" not in st8:
                        st8["ps"] = psP.tile([P, GCOLS], DT, tag="psP",
                                             name=f"psv{qi}{s}")
                    ps = st8["ps"]
                    for i in range(i0, i0 + n):
                        nc.tensor.matmul(
                            ps,
                            lhsT=x_all[:, bass.ds(8 * CH * qi + CH * i + P * s, P)],
                            rhs=wvbig[:, bass.ds(GCOLS * i, GCOLS)],
                            start=(i == 0), stop=(i == 7))
                return go

            def evac():
                dst = vtbig[:, bass.ds(VW * ti, VW)].rearrange(
                    "p (h d) -> p h d", h=HPC)[:, :, 0:HEAD]
                nc.vector.tensor_copy(dst, st8["ps"].rearrange(
                    "p (h d) -> p h d", h=HPC))
            return [(460, mm(0, 4)), (460, mm(4, 4)), (0, evac)]

        def make_o_pset(qi, m, eng, pool=None, ev=None, ysb_pair=None):
            sl = bass.ds(CH * qi, CH)
            st8 = {}
            pool = pool or psP

            def mm():
                if pool is stP:
                    t = pool.tile([P, 2 * CH], DT, tag="stP", name=f"pso{qi}{m}")
                    st8["ps"] = t[:, 0:CH]
                else:
                    st8["ps"] = pool.tile([P, CH], DT, tag="psP",
                                          name=f"pso{qi}{m}")
                for c in range(2):
                    for k in range(2):
                        nc.tensor.matmul(
                            st8["ps"][:, bass.ds(256 * c, 256)],
                            lhsT=wobig[:, bass.ds(EMBED * k + P * m, P)],
                            rhs=ohat_t[k][:, bass.ds(CH * qi + 256 * c, 256)],
                            start=(k == 0), stop=(k == 1))

            def evac():
                # m-pairs share one staging tile; the odd member DMAs both
                # halves in a single transfer (half the DMAs and semaphores)
                if m % 2 == 0:
                    ysb_pair["t"] = ypool.tile([P, 2 * CH], DTH, tag="ysb",
                                               name=f"ysb{qi}{m}")
                ysb = ysb_pair["t"]
                half = ysb[:, bass.ds(CH * (m % 2), CH)]
                if ev is nc.scalar:
                    nc.scalar.copy(half, st8["ps"])
                else:
                    nc.vector.tensor_copy(half, st8["ps"])
                if m % 2 == 1:
                    eng.dma_start(
                        out=yT.rearrange("(j q p) s -> p j q s",
                                         q=2, p=P)[:, m // 2, :, sl],
                        in_=ysb.rearrange("p (q s) -> p q s", q=2))
            return [(470, mm), (0, evac)]

        def queue_qkv_chunk(qi, qk_only=False):
            for f in range(4):
                chunk_q.extend(make_qk_pset(qi, f))
            if not qk_only:
                for s in range(4):
                    chunk_q.extend(make_v_pset(qi, s))

        def queue_v_chunk(qi):
            for s in range(4):
                chunk_q.extend(make_v_pset(qi, s))

        def queue_oproj(qi, pool=None):
            ysb_pair = {}
            for m in range(8):
                if pool is stP:
                    # tail: keep the output DMAs on the fast hardware DGEs
                    eng = nc.sync if m % 2 == 0 else nc.scalar
                    ev = nc.scalar if m % 2 == 1 else None
                else:
                    # sync only: gpsimd's software DGE is slow and its queue
                    # serves the normalize-critical broadcasts; scalar paces
                    # the attention exps
                    eng = nc.sync
                    ev = None
                oproj_q.extend(make_o_pset(qi, m, eng, pool=pool, ev=ev,
                                           ysb_pair=ysb_pair))

        def fill(budget):
            while budget > 0 and (chunk_q or oproj_q):
                q = chunk_q if chunk_q else oproj_q
                cost, fn = q.popleft()
                fn()
                budget -= max(cost, 120)

        def flush(q):
            while q:
                _, fn = q.popleft()
                fn()

        # ---- attention pair: pipelined window loop ----
        LAG = 2

        def attn_pair(hp, qi):
            nk = 4 * qi + 4
            po = [psO.tile([P, CH], DT, tag="psO", name=f"po{hp}{qi}{hh}")
                  for hh in range(2)]
            sts = {}
            for w in range(nk + LAG):
                ki = w
                if ki < nk:
                    kr = ki - 4 * qi
                    trim = P * kr if kr >= 0 else 0
                    W = CH - trim
                    pst = stP.tile([P, 2 * CH], DT, tag="stP", name="pst")
                    for hh in range(2):
                        r0 = HEAD * hh
                        nc.tensor.matmul(
                            pst[:, bass.ds(CH * hh + trim, W)],
                            lhsT=kt_t[hp][r0:r0 + HEAD, bass.ds(P * ki, P)],
                            rhs=qt_t[hp][r0:r0 + HEAD, bass.ds(CH * qi + trim, W)],
                            start=True, stop=True)
                    st = stpool.tile([P, 2 * CH], DTB, tag="st", name="st")
                    psrc = pst.rearrange("p (h c) -> p h c", h=2)[:, :, trim:CH]
                    sdst = st.rearrange("p (h c) -> p h c", h=2)[:, :, trim:CH]
                    nc.scalar.activation(sdst, psrc, EXP, scale=0.125)
                    if kr >= 0:
                        for hh in range(2):
                            nc.vector.tensor_mul(
                                st[:, bass.ds(CH * hh + trim, W)],
                                st[:, bass.ds(CH * hh + trim, W)],
                                mask_sb[:, 0:W])
                    sts[ki] = (st, trim)
                    act_ns = (2 * W + 352) * 0.833
                else:
                    act_ns = 1200
                # AV lags QK by LAG windows; the first AV slips one extra
                # window (caught up at w=LAG+1) so the previous pair's
                # normalize has freed the psO ring before AV[0] queues.
                if w in (LAG, LAG + 1):
                    kavs = []
                elif w == LAG + 2:
                    kavs = [0, 1, 2][:nk]
                else:
                    kavs = [w - LAG]
                av_ns = 0
                for kav in kavs:
                    if not (0 <= kav < nk):
                        continue
                    st, trim = sts.pop(kav)
                    W = CH - trim
                    for hh in range(2):
                        h = 2 * hp + hh
                        nc.tensor.matmul(
                            po[hh][0:HEAD + 1, trim:CH],
                            lhsT=vtbig[:, bass.ds(VW * kav + (HEAD + 1) * h,
                                                  HEAD + 1)],
                            rhs=st[:, bass.ds(CH * hh + trim, W)],
                            start=(kav == 0), stop=(kav == nk - 1))
                    av_ns = 2 * (W * 0.42 + 40)
                qk_ns = (CH - max(0, ki - 4 * qi) * P) * 0.42 + 60 if ki < nk else 0
                fill(act_ns - qk_ns - av_ns + 150)
            # normalize: recip of the ones-row denominator, broadcast, scale.
            # On the last pair the outproj tail waits on this chain: copy the
            # denominators on the (idle) scalar engine and process in column
            # halves so the second half's chain is all that trails the last AV.
            last = (hp == 1 and qi == NCH - 1)
            if not last:
                for hh in range(2):
                    r0 = HEAD * hh
                    den = rpool.tile([1, CH], DT, tag=f"den{hh}", name="den")
                    nc.vector.tensor_copy(den[:], po[hh][HEAD:HEAD + 1, :])
                    recip = rpool.tile([1, CH], DT, tag="recip", name="recip")
                    nc.vector.reciprocal_approx_fast(recip[:], den[:])
                    recipb = rpool.tile([HEAD, CH], DT, tag="recipb", name="recipb")
                    nc.gpsimd.partition_broadcast(recipb[:], recip[:])
                    nc.vector.tensor_mul(
                        ohat_t[hp][r0:r0 + HEAD, bass.ds(CH * qi, CH)],
                        po[hh][0:HEAD, :], recipb[:])
            else:
                # tail chain: fresh tiles (no ring-reuse waits), scalar copies,
                # heads pipelined across scalar/DVE/gpsimd
                dens, recips = [], []
                for hh in range(2):
                    den = tailpool.tile([1, CH], DT, tag=f"tden{hh}", name="tden")
                    nc.scalar.copy(den[:], po[hh][HEAD:HEAD + 1, :])
                    dens.append(den)
                for hh in range(2):
                    recip = tailpool.tile([1, CH], DT, tag=f"trec{hh}", name="trec")
                    nc.vector.reciprocal_approx_fast(recip[:], dens[hh][:])
                    recips.append(recip)
                for hh in range(2):
                    r0 = HEAD * hh
                    recipb = tailpool.tile([HEAD, CH], DT, tag=f"trb{hh}", name="trb")
                    nc.gpsimd.partition_broadcast(recipb[:], recips[hh][:])
                    nc.vector.tensor_mul(
                        ohat_t[hp][r0:r0 + HEAD, bass.ds(CH * qi, CH)],
                        po[hh][0:HEAD, :], recipb[:])

        # ---- schedule ----
        queue_qkv_chunk(0, qk_only=True)
        flush(chunk_q)
        queue_v_chunk(0)
        queue_qkv_chunk(1)
        attn_pair(0, 0)
        attn_pair(1, 0)
        flush(chunk_q)
        queue_qkv_chunk(2)
        attn_pair(0, 1)
        attn_pair(1, 1)
        flush(chunk_q)
        queue_qkv_chunk(3)
        queue_oproj(0)
        attn_pair(0, 2)
        attn_pair(1, 2)
        flush(chunk_q)
        queue_oproj(1)
        queue_oproj(2)
        attn_pair(0, 3)
        attn_pair(1, 3)
        flush(oproj_q)
        queue_oproj(3, pool=stP)
        flush(oproj_q)

    nc.compile()
    return nc


def _make_mask():
    p = np.arange(P)[:, None]
    j = np.arange(CH)[None, :]
    return (p <= j).astype(np.float32)


def kernel(x, W_qkv, b_qkv, W_out, b_out):
    global LAST_EXEC_NS, LAST_RESULTS
    x = np.asarray(x, dtype=np.float32)
    W_qkv = np.asarray(W_qkv, dtype=np.float32)
    b_qkv = np.asarray(b_qkv, dtype=np.float32)
    W_out = np.asarray(W_out, dtype=np.float32)
    b_out = np.asarray(b_out, dtype=np.float32)

    nc = _build_program()
    mask = _make_mask()

    in_maps = []
    for c in range(N_CORES):
        b, g = divmod(c, HPC)
        q0 = GCOLS * g
        wq = W_qkv[q0:q0 + GCOLS]                    # [256, 1024]
        wk = W_qkv[EMBED + q0:EMBED + q0 + GCOLS]
        wv = W_qkv[2 * EMBED + q0:2 * EMBED + q0 + GCOLS]
        bq = b_qkv[q0:q0 + GCOLS]
        bk = b_qkv[EMBED + q0:EMBED + q0 + GCOLS]
        bqk = np.stack([bq[0:P], bq[P:2 * P], bk[0:P], bk[P:2 * P]],
                       axis=1).astype(np.float32)   # [128, 4]
        wqkt = np.concatenate([wq, wk], 0).T            # [1024 embed, 512]
        wqkf = wqkt.reshape(8, P, 4, P).transpose(1, 2, 0, 3).reshape(P, 4 * 8 * P)
        # xP[p, qi, i, s]: chunk-major so each chunk is one contiguous DMA
        xp = x[b].T.reshape(8, P, NCH, CH).transpose(1, 2, 0, 3).reshape(P, 8 * SEQ)
        # wvF in wvbig layout [p, i, 256]
        wvf = wv.T.reshape(8, P, GCOLS).transpose(1, 0, 2).reshape(P, 8 * GCOLS)
        # woF in wobig layout [p, k, 1024]
        wof = W_out[:, q0:q0 + GCOLS].T.reshape(2, P, EMBED).transpose(
            1, 0, 2).reshape(P, 2 * EMBED)
        in_maps.append({
            "xP": np.ascontiguousarray(xp).astype(ml_dtypes.bfloat16),
            "wqkF": np.ascontiguousarray(wqkf).astype(ml_dtypes.bfloat16),
            "wvF": np.ascontiguousarray(wvf).astype(ml_dtypes.bfloat16),
            "bqk": np.ascontiguousarray(bqk),
            "woF": np.ascontiguousarray(wof).astype(ml_dtypes.bfloat16),
            "maskT": mask.astype(ml_dtypes.bfloat16),
        })

    want_trace = bool(int(os.environ.get("KTRACE", "0")))
    if want_trace:
        try:
            import antenv.axon_hooks  # noqa: F401
        except ImportError:
            want_trace = False
    res = run_bass_kernel_spmd(nc, in_maps, list(range(N_CORES)),
                               trace=want_trace,
                               tmpdir=os.environ.get("KTRACE_DIR") or None)
    LAST_EXEC_NS = res.exec_time_ns
    LAST_RESULTS = res

    out = np.empty((BATCH, SEQ, EMBED), dtype=np.float32)
    crow = (b_out + W_out @ b_qkv[2 * EMBED:]).astype(np.float32)
    for b in range(BATCH):
        acc = np.zeros((EMBED, SEQ), dtype=np.float32)
        for g in range(HPC):
            acc += res.results[HPC * b + g]["yT"].astype(np.float32)
        out[b] = acc.T + crow[None, :]
    return out
